# revision 29
# baseline (speedup 1.0000x reference)
"""Trainium2 Bass kernel for nn_Classifier_64587718197982 (spiking CNN).

Network (per reference):
  3x [conv3x3(C=128, pad=1, no bias) -> TDBN (batchnorm over T,B,H,W) -> LIF]
  -> mean over (H,W) -> mean over T -> FC(128->10)

Sharding: data-parallel over batch B=32 across 8 NeuronCores (4 images/core).
TDBN stats become a tiny per-layer AllGather + local sum.

v2 design (on top of the v1 SBUF-resident pipeline):
  - Layers 2/3 convs run entirely in fp8-e4m3 DoubleRow matmuls (0.5
    cycles/output column).  Spikes are exact in fp8; weights use a
    two-term hi+lo e4m3 split (~2^-8 relative weight error).  The two DR
    contraction halves carry TWO DIFFERENT (tap, term) pairs: spikes live
    in a zero-PADDED [C,34,34] fp8 buffer so every tap is a full-width
    matmul (pad ring = conv zero padding), and the moving AP's half-axis
    stride is the flat offset between the two taps.  9 DR matmuls of 512
    cols per psum half-image -> 2304 PE cycles/image (4x fewer than fp16).
  - LIF runs in 4^t-scaled membrane space U_t = 4^t * u_t, which absorbs
    the 0.25 decay into per-timestep constants: 3 full-size ops per step:
      r:    U <- U * [U <= thp*4^(t-1)]      (one scalar_tensor_tensor)
      U':   U <- (dp*4^t + y4) + U           (one scalar_tensor_tensor)
      pack: x8 <- [U > thp*4^t] * (4^t/64)   (one tensor_scalar, fp8 out)
    The 4^t scale on y comes for free: conv1 drains scale by 4^t (Act
    scale), conv2/3 moving spikes carry 4^t/64 while weights carry x64.
  - Stats: drain accum gives per-image-slot sums of 4^t*y; a host-staged
    4^-t weight vector unscales them at reduce time.  The Square pass
    reads y4 with Act scale 4^-t (squared internally -> unscaled sum(y^2))
    and subsamples columns by K_SQS (default 2).
  - Collective: AllGather of [C,2] + local sum (cheaper than AllReduce).
  - Tail (LIF3) keeps the v1 sign-domain form (gate = 0.5 - 0.5*sgn in
    4^t space) with the DVE/GpSimd split, Sign+accum pooling on Act.
"""
import numpy as np
import ml_dtypes
from contextlib import ExitStack

import concourse.bass as bass
import concourse.mybir as mybir
import concourse.tile as tile
from concourse.ap import AP
from concourse import bass_isa
from concourse import bacc
from concourse.bass_utils import run_bass_kernel_spmd

F32 = mybir.dt.float32
FP16 = mybir.dt.float16
FP8E5 = mybir.dt.float8e5
FP8E4 = mybir.dt.float8e4
AF = mybir.ActivationFunctionType
ALU = mybir.AluOpType

T, B, C, H, W = 8, 32, 128, 32, 32
NCORES = 8
BL = B // NCORES          # images per core per timestep
HW = H * W                # 1024
NIMG = T * BL             # 32 images per core
DECAY = 0.25
THRESH = 0.5
BN_EPS = 1e-5
POOL_N = float(T * HW)    # pooling divisor 8192
HP, WP = H + 2, W + 2     # padded spike image
HWP = HP * WP             # 1156
WS = 64.0                 # weight scale for e4m3 two-term split
import os as _os
SIM1 = _os.environ.get("K_SIM1", "") == "1"  # single-core sim (no collectives)
NOAR = _os.environ.get("K_NOAR", "") == "1"  # timing probe: skip collective (wrong results)
AG = _os.environ.get("K_AG", "1") == "1"     # AllGather+local sum vs AllReduce
SQS = int(_os.environ.get("K_SQS", "1"))     # square-pass column stride
PACK_POOL = _os.environ.get("K_PACK_POOL", "1") == "1"  # pack op on gpsimd
# how many of the 4 per-t images run the LIF reset (r) op on gpsimd
RPOOL = int(_os.environ.get("K_RPOOL", "1"))
DBG = _os.environ.get("K_DBG", "") == "1"   # dump intermediates to DRAM

# layer-1 term scheme: "dr" = fp16 main + e5m2 DoubleRow corrections;
# integers fall back to the fp16 multi-term schemes (1/2/3)
L1_MODE = _os.environ.get("K_L1", "dr")
TERMS_L1 = 0 if L1_MODE == "dr" else int(L1_MODE)
# e5m2 correction-term balance scale (wlo*S paired with x/S; whi/S with xlo*S)
DR_S = 64.0
# psum tile columns (1024 = full image, 2 banks per tile)
PSUM_COLS = int(_os.environ.get("K_PSUM_COLS", "1024"))
NHALF = HW // PSUM_COLS
RH = H // NHALF
SLOTS = NIMG * NHALF

# conv shift order: (1,1) first so the start=True matmul covers the full tile
SHIFTS = [(1, 1)] + [(dy, dx) for dy in range(3) for dx in range(3)
                     if not (dy == 1 and dx == 1)]

# tap-pair table for the DR convs of layers 2/3: each matmul's two
# contraction halves are (tap, term) pairs; term 0 = hi = e4m3(w*WS),
# term 1 = lo = e4m3(w*WS - hi).  flat(k) = (k//3)*WP + k%3; the moving
# AP half-axis stride is flat(kb) - flat(ka) (all pairs chosen positive).
PAIRS = [((0, 0), (2, 0)), ((3, 0), (5, 0)), ((6, 0), (8, 0)),
         ((0, 1), (2, 1)), ((3, 1), (5, 1)), ((6, 1), (8, 1)),
         ((1, 0), (7, 0)), ((1, 1), (4, 0)), ((4, 1), (7, 1))]


def _flat(k):
    return (k // 3) * WP + (k % 3)


def _conv_image(nc, psum_pool, y_img, terms, dr_terms=(), acc=None,
                drain_scale=1.0):
    """Layer-1 conv for one image (fp16 main + e5m2 DR corrections).
    Drains PSUM (Act, scale=drain_scale); accum_out gives sum(scale*y);
    a second Act pass (Square, scale=1/drain_scale) accumulates sum(y^2)
    over a 1/SQS column subsample."""
    for h in range(NHALF):
        r_base = h * RH
        pt = psum_pool.tile([C, RH * W], F32, tag="pt")
        p3 = pt.rearrange("c (r w) -> c r w", r=RH)
        # collect all matmuls (16-row groups: psum accumulation group and
        # fp8 moving operand are both capped at 512 output cols)
        mms = []
        for (dy, dx) in SHIFTS:
            oy, ox = dy - 1, dx - 1
            r0 = max(r_base, -oy)
            r1 = min(r_base + RH, H - oy)
            c0 = max(0, -ox)
            c1 = min(W, W - ox)
            k = dy * 3 + dx
            for q0 in range(r_base, r_base + RH, 16):
                s0 = max(q0, r0)
                s1 = min(min(q0 + 16, r_base + RH), r1)
                if s1 <= s0:
                    continue
                # tap (1,1) is emitted first and fully covers each 16-row
                # group: its MMs carry start=True (PSUM reset per word)
                st = (dy, dx) == (1, 1)
                for (w_sb, x3d) in terms:
                    mms.append((
                        p3[:, s0 - r_base:s1 - r_base, c0:c1],
                        w_sb[:, k, :],
                        x3d[:, s0 + oy:s1 + oy, c0 + ox:c1 + ox],
                        None, st and (w_sb, x3d) == terms[0]))
                for (w8, x8) in dr_terms:
                    mms.append((
                        p3[:, s0 - r_base:s1 - r_base, c0:c1],
                        w8[:, k],
                        x8[:, :, s0 + oy:s1 + oy, c0 + ox:c1 + ox],
                        mybir.MatmulPerfMode.DoubleRow, False))
        for i_mm, (out_ap, w_ap, x_ap, pm, st) in enumerate(mms):
            nc.tensor.matmul(out_ap, w_ap, x_ap,
                             start=st, stop=(i_mm == len(mms) - 1),
                             perf_mode=pm)
        _drain(nc, y_img, pt, h, acc, drain_scale)


def _drain(nc, y_img, pt, h, acc, drain_scale, y_scale=None):
    """Drain PSUM -> y (Act Copy, scale=drain_scale; accum -> sum slot) and
    accumulate sum(y_true^2) via Square with scale 1/y_scale, where y_scale
    is the TOTAL 4^t factor of the stored y (drain_scale for conv1, where
    the scale enters at the drain; 4^t for conv2/3, where it rides in the
    fp8 moving operand)."""
    if y_scale is None:
        y_scale = drain_scale
    y_half = y_img[:, h * PSUM_COLS:(h + 1) * PSUM_COLS]
    if acc is None:
        nc.scalar.activation(y_half, pt, AF.Copy, scale=drain_scale)
        return
    sum_slot, sq_slot, scratch_pool, slot = acc
    nc.scalar.activation(y_half, pt, AF.Copy, scale=drain_scale,
                         accum_out=sum_slot[:, slot + h:slot + h + 1])
    ysub = y_half.rearrange("c (p s) -> c p s", s=SQS)[:, :, 0]
    scr = scratch_pool.tile([C, PSUM_COLS // SQS], FP16, tag="sqscr")
    nc.scalar.activation(scr, ysub, AF.Square, scale=1.0 / y_scale,
                         accum_out=sq_slot[:, slot + h:slot + h + 1])


def _conv_dr(nc, psum_pool, y_img, wpk, x8pad, acc=None, drain_scale=1.0,
             y_scale=1.0):
    """Layer 2/3 conv for one image: tap-paired e4m3 DR matmuls reading the
    zero-padded spike buffer, in 16-row groups (fp8 moving operand is capped
    at 1024 elements = 512 output cols).  The moving operand's 4^t/WS scale
    makes the PSUM result 4^t * y; drain scale stays 1."""
    pstride = x8pad.ap[0][0]
    base_off = x8pad.offset
    n_mm = len(PAIRS) * (RH // 16)
    for h in range(NHALF):
        pt = psum_pool.tile([C, PSUM_COLS], F32, tag="pt")
        p3 = pt.rearrange("c (r w) -> c r w", r=RH)
        i_mm = 0
        for g in range(RH // 16):
            r0 = h * RH + g * 16
            for m, ((ka, _), (kb, _)) in enumerate(PAIRS):
                fa, fb = _flat(ka), _flat(kb)
                mv = AP(x8pad.tensor, base_off + r0 * WP + fa,
                        [[pstride, C], [fb - fa, 2], [WP, 16], [1, W]])
                nc.tensor.matmul(
                    p3[:, g * 16:(g + 1) * 16, :], wpk[:, m], mv,
                    start=(m == 0), stop=(i_mm == n_mm - 1),
                    perf_mode=mybir.MatmulPerfMode.DoubleRow,
                )
                i_mm += 1
        _drain(nc, y_img, pt, h, acc, drain_scale, y_scale)


def _layer_stats(nc, sb1, stats_buf, pre, eps_t, wvec, pow4, cc_in, cc_out,
                 lname):
    """Weighted stats reduce + AllGather (or AllReduce) + u-space
    coefficients, expanded to per-timestep 4^t tiles.

    Returns (dp4, thp4, negthp4): [C, T] tiles, column t = 4^t * coeff."""
    sums, sqs = stats_buf
    cc = sb1.tile([C, 2], F32, tag=f"cc{lname}")
    tmp = sb1.tile([C, SLOTS], F32, tag=f"ccw{lname}")
    nc.vector.tensor_tensor(tmp, sums, wvec, op=ALU.mult)
    nc.vector.tensor_reduce(cc[:, 0:1], tmp,
                            axis=mybir.AxisListType.X, op=ALU.add)
    nc.vector.tensor_reduce(cc[:, 1:2], sqs,
                            axis=mybir.AxisListType.X, op=ALU.add)
    nc.sync.dma_start(out=cc_in[:, :], in_=cc)
    ccs = sb1.tile([C, 2], F32, tag=f"ccs{lname}")
    if SIM1 or NOAR:
        nc.sync.dma_start(out=ccs, in_=cc_in[:, :])
    elif AG:
        nc.gpsimd.collective_compute(
            "AllGather", ALU.bypass,
            replica_groups=[list(range(NCORES))],
            ins=[cc_in[:, :]], outs=[cc_out[:, :]],
        )
        ccr = sb1.tile([C, 2, NCORES], F32, tag=f"ccr{lname}")
        nc.sync.dma_start(
            out=ccr, in_=cc_out[:, :].rearrange("(g c) k -> c k g", g=NCORES))
        nc.vector.tensor_reduce(ccs.unsqueeze(2), ccr,
                                axis=mybir.AxisListType.X, op=ALU.add)
    else:
        nc.gpsimd.collective_compute(
            "AllReduce", ALU.add,
            replica_groups=[list(range(NCORES))],
            ins=[cc_in[:, :]], outs=[cc_out[:C, :]],
        )
        nc.sync.dma_start(out=ccs, in_=cc_out[:C, :])
    inv_n = 1.0 / ((1 if SIM1 else NCORES) * NIMG * HW)
    # short critical-path algebra: th' = 0.5*sd/bnw, d' = bnb*sd/bnw - mean
    rbw, nrbw, bbrb = pre
    mean = sb1.tile([C, 1], F32, tag=f"mean{lname}")
    nc.vector.tensor_scalar(mean, ccs[:, 0:1], inv_n, None, op0=ALU.mult)
    msq = sb1.tile([C, 1], F32, tag=f"msq{lname}")
    nc.vector.tensor_tensor(msq, mean, mean, op=ALU.mult)
    var = sb1.tile([C, 1], F32, tag=f"var{lname}")
    nc.vector.scalar_tensor_tensor(var, ccs[:, 1:2], inv_n * SQS, msq,
                                   op0=ALU.mult, op1=ALU.subtract)
    sd = sb1.tile([C, 1], F32, tag=f"sd{lname}")
    nc.scalar.activation(sd, var, AF.Sqrt, bias=eps_t, scale=1.0)
    thp = sb1.tile([C, 1], F32, tag=f"thp{lname}")
    nc.vector.tensor_tensor(thp, sd, rbw, op=ALU.mult)
    dp = sb1.tile([C, 1], F32, tag=f"dp{lname}")
    nc.vector.scalar_tensor_tensor(dp, sd, bbrb, mean,
                                   op0=ALU.mult, op1=ALU.subtract)
    # expand to per-timestep 4^t columns (one op each via pow4 broadcast)
    thp4 = sb1.tile([C, T], F32, tag=f"thp4{lname}")
    nc.vector.tensor_tensor(thp4, thp.broadcast_to([C, T]), pow4,
                            op=ALU.mult)
    dp4 = sb1.tile([C, T], F32, tag=f"dp4{lname}")
    nc.vector.tensor_tensor(dp4, dp.broadcast_to([C, T]), pow4, op=ALU.mult)
    negthp4 = sb1.tile([C, T], F32, tag=f"negthp4{lname}")
    nc.vector.tensor_scalar(negthp4, thp4, -1.0, None, op0=ALU.mult)
    return dp4, thp4, negthp4


def build():
    nc = bacc.Bacc("TRN2", target_bir_lowering=False, debug=False,
                   num_devices=1 if SIM1 else NCORES)

    # --- I/O ---
    xhi_d = nc.dram_tensor("xhi", [T, BL, C, HW], FP16, kind="ExternalInput")
    xlo_d = xc8_d = w1c8_d = None
    if L1_MODE == "dr":
        xc8_d = nc.dram_tensor("xc8", [T, BL, C, 2, HW], FP8E5,
                               kind="ExternalInput")
        w1c8_d = nc.dram_tensor("w1c8", [C, 9, 2, C], FP8E5,
                                kind="ExternalInput")
    elif TERMS_L1 >= 2:
        xlo_d = nc.dram_tensor("xlo", [T, BL, C, HW], FP16,
                               kind="ExternalInput")
    w_d = {(1, "hi"): nc.dram_tensor("w1hi", [C, 9, C], FP16,
                                     kind="ExternalInput")}
    if TERMS_L1 >= 3:
        w_d[(1, "lo")] = nc.dram_tensor("w1lo", [C, 9, C], FP16,
                                        kind="ExternalInput")
    wpk_d = {}
    for l in (2, 3):
        wpk_d[l] = nc.dram_tensor(f"wpk{l}", [C, 9, 2, C], FP8E4,
                                  kind="ExternalInput")
    bn_d = {}
    for l in (1, 2, 3):
        bn_d[(l, "w")] = nc.dram_tensor(f"bnw{l}", [C, 1], F32,
                                        kind="ExternalInput")
        bn_d[(l, "b")] = nc.dram_tensor(f"bnb{l}", [C, 1], F32,
                                        kind="ExternalInput")
    fcw_d = nc.dram_tensor("fcw", [C, 10], F32, kind="ExternalInput")
    fcb_d = nc.dram_tensor("fcb", [1, 10], F32, kind="ExternalInput")
    wvec_d = nc.dram_tensor("wvec", [C, SLOTS], F32, kind="ExternalInput")
    pow4_d = nc.dram_tensor("pow4", [C, T], F32, kind="ExternalInput")
    out_d = nc.dram_tensor("out", [1, BL * 10], F32, kind="ExternalOutput")

    # --- internal DRAM (collective buffers only) ---
    cc_bufs = {}
    for l in (1, 2, 3):
        cc_bufs[l] = (
            nc.dram_tensor(f"cc_in{l}", [C, 2], F32),
            nc.dram_tensor(f"cc_out{l}", [C * (NCORES if AG else 1), 2], F32,
                           addr_space="Shared"),
        )

    with ExitStack() as ctx:
        tc = ctx.enter_context(tile.TileContext(nc))
        sb1 = ctx.enter_context(tc.tile_pool(name="sb1", bufs=1))
        xpool = ctx.enter_context(tc.tile_pool(name="xpool", bufs=4))
        gate_pool = ctx.enter_context(tc.tile_pool(name="gate", bufs=3))
        stage_pool = ctx.enter_context(tc.tile_pool(name="stage", bufs=3))
        mem_pool = ctx.enter_context(tc.tile_pool(name="mem", bufs=1))
        psum_pool = ctx.enter_context(
            tc.tile_pool(name="psum", bufs=(8 * 512) // PSUM_COLS,
                         space="PSUM"))

        # --- load layer-1 weights only; everything else streams during
        # the conv1 window ---
        w_sb = {}
        for key in ((1, "hi"),) + (((1, "lo"),) if (1, "lo") in w_d else ()):
            w_sb[key] = sb1.tile([C, 9, C], FP16,
                                 name=f"w{key[0]}{key[1]}",
                                 tag=f"w{key[0]}{key[1]}")
            nc.sync.dma_start(out=w_sb[key], in_=w_d[key][:, :, :])
        w1c8_sb = None
        if L1_MODE == "dr":
            w1c8_sb = sb1.tile([C, 9, 2, C], FP8E5, name="w1c8", tag="w1c8")
            nc.sync.dma_start(out=w1c8_sb, in_=w1c8_d[:, :, :, :])

        # Warm-up matmuls: touch each weight tensor once with a fused
        # (self-loading) matmul so the weight-DMA waits attach to a Matmult.
        def warmup_mm(w2d):
            pt = psum_pool.tile([C, PSUM_COLS], F32, tag="pt")
            nc.tensor.matmul(pt[:, 0:1], w2d, w2d[:, 0:1],
                             start=True, stop=True)

        warmup_mm(w_sb[(1, "hi")][:, 4, :])
        if L1_MODE == "dr":
            warmup_mm(w1c8_sb[:, 4, 0, :])

        # persistent SBUF activations: y in f32 (per-image scaled by 4^t)
        ybuf = sb1.tile([C, NIMG, HW], F32, name="ybuf")
        # padded fp8 spike buffers: pad ring stays zero (conv zero-padding);
        # 8 rotating persistent slots (WAR tracked by the tile framework)
        x8bufs = []
        for i in range(8):
            xb = sb1.tile([C, HWP], FP8E4, name=f"x8b{i}", tag=f"x8b{i}")
            nc.vector.memset(xb, 0.0)
            x8bufs.append(xb)

        # L3 pooled spike counts per (t, b)
        poolbuf = sb1.tile([C, BL, T], F32)

        stats = {}
        for l in (1, 2, 3):
            ysum = sb1.tile([C, SLOTS], F32, name=f"ysum{l}",
                            tag=f"sum{l}")
            ysq = sb1.tile([C, SLOTS], F32, name=f"ysq{l}",
                           tag=f"sq{l}")
            stats[l] = (ysum, ysq)
        scratch_pool = ctx.enter_context(tc.tile_pool(name="sqscr", bufs=2))

        # =============== layer 1 conv (standalone window) ===============
        for t in range(T):
            for b in range(BL):
                img = t * BL + b
                xhi = xpool.tile([C, HW], FP16, tag="xin")
                nc.sync.dma_start(out=xhi, in_=xhi_d[t, b, :, :])
                xhi3 = xhi.rearrange("c (h w) -> c h w", h=H)
                terms = [(w_sb[(1, "hi")], xhi3)]
                dr_terms = []
                if L1_MODE == "dr":
                    x8 = xpool.tile([C, 2, HW], FP8E5, tag="x8")
                    nc.sync.dma_start(out=x8, in_=xc8_d[t, b])
                    x84 = x8.rearrange("c two (h w) -> c two h w", h=H)
                    dr_terms.append((w1c8_sb, x84))
                else:
                    if TERMS_L1 >= 3:
                        terms.append((w_sb[(1, "lo")], xhi3))
                    if TERMS_L1 >= 2:
                        xlo = xpool.tile([C, HW], FP16, tag="xin")
                        nc.sync.dma_start(out=xlo, in_=xlo_d[t, b, :, :])
                        xlo3 = xlo.rearrange("c (h w) -> c h w", h=H)
                        terms.append((w_sb[(1, "hi")], xlo3))
                _conv_image(nc, psum_pool, ybuf[:, img], terms, dr_terms,
                            acc=(stats[1][0], stats[1][1], scratch_pool,
                                 NHALF * img),
                            drain_scale=float(4.0 ** t))

        # deferred constants: stream in under the conv1 window
        wpk_sb = {}
        for l in (2, 3):
            wpk_sb[l] = sb1.tile([C, 9, 2, C], FP8E4, name=f"wpk{l}",
                                 tag=f"wpk{l}")
            nc.sync.dma_start(out=wpk_sb[l], in_=wpk_d[l][:, :, :, :])
        bn_sb = {}
        for key, dt_ in bn_d.items():
            bn_sb[key] = sb1.tile([C, 1], F32, name=f"bn{key[1]}{key[0]}",
                                  tag=f"bn{key[1]}{key[0]}")
            nc.sync.dma_start(out=bn_sb[key], in_=dt_[:, :])
        fcw_sb = sb1.tile([C, 10], F32)
        nc.sync.dma_start(out=fcw_sb, in_=fcw_d[:, :])
        fcb_sb = sb1.tile([1, 10], F32)
        nc.sync.dma_start(out=fcb_sb, in_=fcb_d[:, :])
        wvec = sb1.tile([C, SLOTS], F32)
        nc.sync.dma_start(out=wvec, in_=wvec_d[:, :])
        pow4 = sb1.tile([C, T], F32)
        nc.sync.dma_start(out=pow4, in_=pow4_d[:, :])
        eps_t = sb1.tile([C, 1], F32)
        nc.vector.memset(eps_t, BN_EPS)
        # off-critical-path per-layer constants: 0.5/bnw, -0.5/bnw, bnb/bnw
        pre = {}
        for l in (1, 2, 3):
            rb = sb1.tile([C, 1], F32, tag=f"rbw{l}")
            nc.vector.reciprocal(out=rb, in_=bn_sb[(l, "w")])
            rbw = sb1.tile([C, 1], F32, tag=f"rbwth{l}")
            nc.vector.tensor_scalar(rbw, rb, THRESH, None, op0=ALU.mult)
            nrbw = sb1.tile([C, 1], F32, tag=f"nrbw{l}")
            nc.vector.tensor_scalar(nrbw, rb, -THRESH, None, op0=ALU.mult)
            bbrb = sb1.tile([C, 1], F32, tag=f"bbrb{l}")
            nc.vector.tensor_tensor(bbrb, bn_sb[(l, "b")], rb, op=ALU.mult)
            pre[l] = (rbw, nrbw, bbrb)
        # fp8 DR warm-ups for the deferred weights (moving data: zeroed x8)
        x8v0 = x8bufs[0].rearrange("c (h w) -> c h w", h=HP)
        for l in (2, 3):
            pt = psum_pool.tile([C, PSUM_COLS], F32, tag="pt")
            nc.tensor.matmul(pt[:, 0:1], wpk_sb[l][:, 4], x8v0[:, 0:2, 0:1],
                             start=True, stop=True,
                             perf_mode=mybir.MatmulPerfMode.DoubleRow)

        dp4_1, thp4_1, _ = _layer_stats(nc, sb1, stats[1], pre[1], eps_t,
                                        wvec, pow4, *cc_bufs[1], "l1")

        # ====== windows 2&3: LIF_l (DVE) interleaved with conv_{l+1} ======
        def lif_conv_window(dp4, thp4, wpk, stats_next):
            """Per image: 4^t-space LIF (r, U', pack) then the DR conv of
            the next layer reading the packed padded spikes."""
            u = mem_pool.tile([C, BL * HW], F32, tag="mem")
            u4 = u.rearrange("c (b p) -> c b p", b=BL)
            for t in range(T):
                # moving fp8 values must stay <= 2^6 (hw e4m3 treats biased
                # exponent 15 as inf/nan): cap the pack scale at 4^6/WS and
                # recover the remaining factor in the drain scale
                pk_scale = float(4.0 ** min(t, 6) / WS)
                ds = float(4.0 ** t / (4.0 ** min(t, 6)))
                for b in range(BL):
                    img = t * BL + b
                    yt = ybuf[:, img]
                    if t == 0:
                        nc.vector.tensor_scalar(u4[:, b], yt,
                                                dp4[:, 0:1], None,
                                                op0=ALU.add)
                    else:
                        # r: U <- U * [U <= thp4[t-1]]  (hard reset)
                        nc.vector.scalar_tensor_tensor(
                            u4[:, b], u4[:, b], thp4[:, t - 1:t], u4[:, b],
                            op0=ALU.is_le, op1=ALU.mult)
                        # U': U <- (y4 + dp4[t]) + U
                        nc.vector.scalar_tensor_tensor(
                            u4[:, b], yt, dp4[:, t:t + 1], u4[:, b],
                            op0=ALU.add, op1=ALU.add)
                    x8 = x8bufs[img % 8]
                    x8int = x8.rearrange("c (h w) -> c h w",
                                         h=HP)[:, 1:H + 1, 1:W + 1]
                    eng = nc.gpsimd if PACK_POOL else nc.vector
                    eng.tensor_scalar(
                        x8int, u4[:, b].rearrange("c (h w) -> c h w", h=H),
                        thp4[:, t:t + 1], pk_scale,
                        op0=ALU.is_gt, op1=ALU.mult)
                    _conv_dr(nc, psum_pool, ybuf[:, img], wpk, x8,
                             acc=(stats_next[0], stats_next[1],
                                  scratch_pool, NHALF * img),
                             drain_scale=ds, y_scale=float(4.0 ** t))

        lif_conv_window(dp4_1, thp4_1, wpk_sb[2], stats[2])
        dp4_2, thp4_2, _ = _layer_stats(nc, sb1, stats[2], pre[2], eps_t,
                                        wvec, pow4, *cc_bufs[2], "l2")

        lif_conv_window(dp4_2, thp4_2, wpk_sb[3], stats[3])
        dp4_3, thp4_3, negthp4_3 = _layer_stats(nc, sb1, stats[3], pre[3],
                                                eps_t, wvec, pow4,
                                                *cc_bufs[3], "l3")

        # =============== layer 3 LIF (4^t space) ===============
        # b 0/1: pure-DVE r-form (reset mask folded into one stt; no
        # cross-engine dependency in the recurrence — the Act Sign is a
        # pooling-only observer).  b 2/3: sign-form with gate+mult on
        # GpSimd (float-scalar ops only; stt is DVE-only in the ISA).
        u = mem_pool.tile([C, BL * HW], F32, tag="mem")
        u4 = u.rearrange("c (b p) -> c b p", b=BL)
        prev_sgn = [None] * BL
        for t in range(T):
            for b in range(BL):
                img = t * BL + b
                yt = ybuf[:, img]
                if t == 0:
                    nc.vector.tensor_scalar(u4[:, b], yt, dp4_3[:, 0:1],
                                            None, op0=ALU.add)
                else:
                    if b < 2:
                        nc.vector.scalar_tensor_tensor(
                            u4[:, b], u4[:, b], thp4_3[:, t - 1:t], u4[:, b],
                            op0=ALU.is_le, op1=ALU.mult)
                    else:
                        gate = gate_pool.tile([C, HW], FP16, tag="gate")
                        nc.gpsimd.tensor_scalar(gate, prev_sgn[b], -0.5, 0.5,
                                                op0=ALU.mult, op1=ALU.add)
                        nc.gpsimd.tensor_tensor(u4[:, b], u4[:, b], gate,
                                                op=ALU.mult)
                    nc.vector.scalar_tensor_tensor(
                        u4[:, b], yt, dp4_3[:, t:t + 1], u4[:, b],
                        op0=ALU.add, op1=ALU.add)
                st = stage_pool.tile([C, HW], FP16, tag="stage")
                nc.scalar.activation(st, u4[:, b], AF.Sign,
                                     bias=negthp4_3[:, t:t + 1], scale=1.0,
                                     accum_out=poolbuf[:, b, t:t + 1])
                prev_sgn[b] = st

        # =============== head: pooling + FC ===============
        feat = sb1.tile([C, BL], F32)
        nc.vector.tensor_reduce(feat.unsqueeze(2), poolbuf,
                                axis=mybir.AxisListType.X, op=ALU.add)
        nc.vector.tensor_scalar(feat, feat, 0.5, T * HW / 2.0,
                                op0=ALU.mult, op1=ALU.add)
        prod = sb1.tile([C, BL, 10], F32)
        nc.vector.tensor_tensor(
            prod, feat.unsqueeze(2).broadcast_to([C, BL, 10]),
            fcw_sb.unsqueeze(1).broadcast_to([C, BL, 10]), op=ALU.mult)
        red = sb1.tile([C, BL, 10], F32)
        nc.gpsimd.partition_all_reduce(red, prod, channels=C,
                                       reduce_op=bass_isa.ReduceOp.add)
        ofin = sb1.tile([1, BL, 10], F32)
        nc.vector.tensor_scalar(ofin, red[0:1], 1.0 / POOL_N, None,
                                op0=ALU.mult)
        nc.vector.tensor_tensor(
            ofin, ofin, fcb_sb.unsqueeze(1).broadcast_to([1, BL, 10]),
            op=ALU.add)
        nc.sync.dma_start(out=out_d[:, :],
                          in_=ofin.rearrange("c b k -> c (b k)"))

        if DBG:
            d_ybuf = nc.dram_tensor("d_ybuf", [C, NIMG * HW], F32,
                                    kind="ExternalOutput")
            nc.sync.dma_start(out=d_ybuf[:, :],
                              in_=ybuf.rearrange("c n p -> c (n p)"))
            d_pool = nc.dram_tensor("d_pool", [C, BL * T], F32,
                                    kind="ExternalOutput")
            nc.sync.dma_start(out=d_pool[:, :],
                              in_=poolbuf.rearrange("c b t -> c (b t)"))
            for l in (1, 2, 3):
                ds = nc.dram_tensor(f"d_sum{l}", [C, SLOTS], F32,
                                    kind="ExternalOutput")
                nc.sync.dma_start(out=ds[:, :], in_=stats[l][0])
                dq = nc.dram_tensor(f"d_sq{l}", [C, SLOTS], F32,
                                    kind="ExternalOutput")
                nc.sync.dma_start(out=dq[:, :], in_=stats[l][1])
            for i in range(8):
                dx8 = nc.dram_tensor(f"d_x8_{i}", [C, HWP], FP8E4,
                                     kind="ExternalOutput")
                nc.sync.dma_start(out=dx8[:, :], in_=x8bufs[i])
            for l, t4 in (("dp1", dp4_1), ("th1", thp4_1),
                          ("dp3", dp4_3), ("nth3", negthp4_3)):
                dt4 = nc.dram_tensor(f"d_{l}", [C, T], F32,
                                     kind="ExternalOutput")
                nc.sync.dma_start(out=dt4[:, :], in_=t4)

    # walrus rejects the standalone InstLdweights this pass splits out for
    # our multi-wait first-of-image matmuls; excess waits lower to
    # event-semaphore chains instead.
    nc.move_matmul_waits_to_ldweights = lambda: None
    nc.compile()
    return nc


_NC_CACHE = {}


def _get_nc():
    if "nc" not in _NC_CACHE:
        _NC_CACHE["nc"] = build()
    return _NC_CACHE["nc"]


def _hi_lo(a):
    hi = a.astype(np.float16)
    lo = (a - hi.astype(np.float32)).astype(np.float16)
    return hi, lo


def _e5m2(a, scale):
    return (np.asarray(a, np.float32) * scale).astype(ml_dtypes.float8_e5m2)


def _e4m3(a):
    return np.asarray(a, np.float32).astype(ml_dtypes.float8_e4m3fn)


def make_in_maps(inp, conv_ws, bns, fc_w, fc_b):
    """Build the 8 per-core input maps from full (numpy) model inputs."""
    common = {}
    for li, w in enumerate(conv_ws, start=1):
        wt = np.ascontiguousarray(
            w.transpose(1, 2, 3, 0).reshape(C, 9, C))  # [I, k, O]
        if li == 1:
            hi, lo = _hi_lo(wt)
            common["w1hi"] = hi
            if L1_MODE == "dr":
                w1c8 = np.empty((C, 9, 2, C), dtype=ml_dtypes.float8_e5m2)
                w1c8[:, :, 0, :] = _e5m2(lo.astype(np.float32), DR_S)
                w1c8[:, :, 1, :] = _e5m2(hi.astype(np.float32), 1.0 / DR_S)
                common["w1c8"] = w1c8
            if TERMS_L1 >= 3:
                common["w1lo"] = lo
        else:
            ws_hi = _e4m3(wt * WS)
            ws_lo = _e4m3(wt * WS - ws_hi.astype(np.float32))
            terms = (ws_hi, ws_lo)
            wpk = np.empty((C, 9, 2, C), dtype=ml_dtypes.float8_e4m3fn)
            for m, ((ka, ta), (kb, tb)) in enumerate(PAIRS):
                wpk[:, m, 0, :] = terms[ta][:, ka, :]
                wpk[:, m, 1, :] = terms[tb][:, kb, :]
            common[f"wpk{li}"] = wpk
        common[f"bnw{li}"] = np.ascontiguousarray(
            bns[li - 1][0].reshape(C, 1))
        common[f"bnb{li}"] = np.ascontiguousarray(
            bns[li - 1][1].reshape(C, 1))
    common["fcw"] = np.ascontiguousarray(fc_w.T)          # [C, 10]
    common["fcb"] = np.ascontiguousarray(fc_b.reshape(1, 10))
    wvec = np.zeros((1, SLOTS), np.float32)
    for t in range(T):
        for b in range(BL):
            img = t * BL + b
            for h in range(NHALF):
                wvec[0, NHALF * img + h] = 4.0 ** (-t)
    common["wvec"] = np.ascontiguousarray(np.broadcast_to(wvec, (C, SLOTS)))
    pow4 = np.asarray([[4.0 ** t for t in range(T)]], np.float32)
    common["pow4"] = np.ascontiguousarray(np.broadcast_to(pow4, (C, T)))

    in_maps = []
    for cid in range(NCORES):
        xc = np.ascontiguousarray(
            inp[:, cid * BL:(cid + 1) * BL].reshape(T, BL, C, HW))
        xhi, xlo = _hi_lo(xc)
        m = dict(common)
        m["xhi"] = xhi
        if L1_MODE == "dr":
            xc8 = np.empty((T, BL, C, 2, HW), dtype=ml_dtypes.float8_e5m2)
            xc8[:, :, :, 0, :] = _e5m2(xc, 1.0 / DR_S)
            xc8[:, :, :, 1, :] = _e5m2(xlo.astype(np.float32), DR_S)
            m["xc8"] = xc8
        elif TERMS_L1 >= 2:
            m["xlo"] = xlo
        in_maps.append(m)
    return in_maps


def kernel(inp, conv_w1, conv_w2, conv_w3, bn_w1, bn_b1, bn_w2, bn_b2,
           bn_w3, bn_b3, fc_w, fc_b):
    inp = np.asarray(inp, dtype=np.float32)
    ws = [np.asarray(w, dtype=np.float32) for w in (conv_w1, conv_w2, conv_w3)]
    bns = [(np.asarray(bn_w1, np.float32), np.asarray(bn_b1, np.float32)),
           (np.asarray(bn_w2, np.float32), np.asarray(bn_b2, np.float32)),
           (np.asarray(bn_w3, np.float32), np.asarray(bn_b3, np.float32))]
    fc_w = np.asarray(fc_w, np.float32)
    fc_b = np.asarray(fc_b, np.float32)

    nc = _get_nc()
    in_maps = make_in_maps(inp, ws, bns, fc_w, fc_b)
    res = run_bass_kernel_spmd(nc, in_maps, core_ids=list(range(NCORES)))
    out = np.concatenate(
        [r["out"].reshape(BL, 10) for r in res.results], axis=0)
    return out.astype(np.float32)


# revision 37
# speedup vs baseline: 3.3925x; 3.3925x over previous
"""Trainium2 Bass kernel for nn_Classifier_64587718197982 (spiking CNN).

Network (per reference):
  3x [conv3x3(C=128, pad=1, no bias) -> TDBN (batchnorm over T,B,H,W) -> LIF]
  -> mean over (H,W) -> mean over T -> FC(128->10)

Sharding: data-parallel over batch B=32 across 8 NeuronCores (4 images/core).
TDBN stats become a tiny per-layer AllGather + local sum.

v2 design (on top of the v1 SBUF-resident pipeline):
  - Layers 2/3 convs run entirely in fp8-e4m3 DoubleRow matmuls (0.5
    cycles/output column).  Spikes are exact in fp8; weights use a
    two-term hi+lo e4m3 split (~2^-8 relative weight error).  The two DR
    contraction halves carry TWO DIFFERENT (tap, term) pairs: spikes live
    in a zero-PADDED [C,34,34] fp8 buffer so every tap is a full-width
    matmul (pad ring = conv zero padding), and the moving AP's half-axis
    stride is the flat offset between the two taps.  9 DR matmuls of 512
    cols per psum half-image -> 2304 PE cycles/image (4x fewer than fp16).
  - LIF runs in 4^t-scaled membrane space U_t = 4^t * u_t, which absorbs
    the 0.25 decay into per-timestep constants: 3 full-size ops per step:
      r:    U <- U * [U <= thp*4^(t-1)]      (one scalar_tensor_tensor)
      U':   U <- (dp*4^t + y4) + U           (one scalar_tensor_tensor)
      pack: x8 <- [U > thp*4^t] * (4^t/64)   (one tensor_scalar, fp8 out)
    The 4^t scale on y comes for free: conv1 drains scale by 4^t (Act
    scale), conv2/3 moving spikes carry 4^t/64 while weights carry x64.
  - Stats: drain accum gives per-image-slot sums of 4^t*y; a host-staged
    4^-t weight vector unscales them at reduce time.  The Square pass
    reads y4 with Act scale 4^-t (squared internally -> unscaled sum(y^2))
    and subsamples columns by K_SQS (default 2).
  - Collective: AllGather of [C,2] + local sum (cheaper than AllReduce).
  - Tail (LIF3) keeps the v1 sign-domain form (gate = 0.5 - 0.5*sgn in
    4^t space) with the DVE/GpSimd split, Sign+accum pooling on Act.
"""
import numpy as np
import ml_dtypes
from contextlib import ExitStack

import concourse.bass as bass
import concourse.mybir as mybir
import concourse.tile as tile
from concourse.ap import AP
from concourse import bass_isa
from concourse import bacc
from concourse.bass_utils import run_bass_kernel_spmd

F32 = mybir.dt.float32
F32R = mybir.dt.float32r
FP16 = mybir.dt.float16
FP8E5 = mybir.dt.float8e5
FP8E4 = mybir.dt.float8e4
AF = mybir.ActivationFunctionType
ALU = mybir.AluOpType

T, B, C, H, W = 8, 32, 128, 32, 32
NCORES = 8
BL = B // NCORES          # images per core per timestep
HW = H * W                # 1024
NIMG = T * BL             # 32 images per core
DECAY = 0.25
THRESH = 0.5
BN_EPS = 1e-5
POOL_N = float(T * HW)    # pooling divisor 8192
HP, WP = H + 2, W + 2     # padded spike image
HWP = HP * WP             # 1156
WS = 64.0                 # weight scale for e4m3 two-term split
import os as _os
SIM1 = _os.environ.get("K_SIM1", "") == "1"  # single-core sim (no collectives)
NOAR = _os.environ.get("K_NOAR", "") == "1"  # timing probe: skip collective (wrong results)
AG = _os.environ.get("K_AG", "1") == "1"     # AllGather+local sum vs AllReduce
SQS = int(_os.environ.get("K_SQS", "1"))     # square-pass column stride
PACK_POOL = _os.environ.get("K_PACK_POOL", "1") == "1"  # pack op on gpsimd
# how many of the 4 per-t images run the LIF reset (r) op on gpsimd
RPOOL = int(_os.environ.get("K_RPOOL", "1"))
DBG = _os.environ.get("K_DBG", "") == "1"   # dump intermediates to DRAM

# layer-1 term scheme: "f32r" = single float32r term (1 cyc/col at
# free>=256, ~2x more accurate than fp16); "dr" = fp16 main + e5m2
# DoubleRow corrections; integers = fp16 multi-term schemes (1/2/3)
L1_MODE = _os.environ.get("K_L1", "f32r")
TERMS_L1 = 0 if L1_MODE in ("dr", "f32r") else int(L1_MODE)
# e5m2 correction-term balance scale (wlo*S paired with x/S; whi/S with xlo*S)
DR_S = 64.0
# psum tile columns (1024 = full image, 2 banks per tile)
PSUM_COLS = int(_os.environ.get("K_PSUM_COLS", "1024"))
NHALF = HW // PSUM_COLS
RH = H // NHALF
SLOTS = NIMG * NHALF

# conv shift order: (1,1) first so the start=True matmul covers the full tile
SHIFTS = [(1, 1)] + [(dy, dx) for dy in range(3) for dx in range(3)
                     if not (dy == 1 and dx == 1)]

# tap-pair table for the DR convs of layers 2/3: each matmul's two
# contraction halves are (tap, term) pairs; term 0 = hi = e4m3(w*WS),
# term 1 = lo = e4m3(w*WS - hi).  flat(k) = (k//3)*WP + k%3; the moving
# AP half-axis stride is flat(kb) - flat(ka) (all pairs chosen positive).
PAIRS = [((0, 0), (2, 0)), ((3, 0), (5, 0)), ((6, 0), (8, 0)),
         ((0, 1), (2, 1)), ((3, 1), (5, 1)), ((6, 1), (8, 1)),
         ((1, 0), (7, 0)), ((1, 1), (4, 0)), ((4, 1), (7, 1))]


def _flat(k):
    return (k // 3) * WP + (k % 3)


def _conv_image(nc, psum_pool, y_img, terms, dr_terms=(), acc=None,
                drain_scale=1.0):
    """Layer-1 conv for one image (fp16 main + e5m2 DR corrections).
    Drains PSUM (Act, scale=drain_scale); accum_out gives sum(scale*y);
    a second Act pass (Square, scale=1/drain_scale) accumulates sum(y^2)
    over a 1/SQS column subsample."""
    for h in range(NHALF):
        r_base = h * RH
        pt = psum_pool.tile([C, RH * W], F32, tag="pt")
        p3 = pt.rearrange("c (r w) -> c r w", r=RH)
        # collect all matmuls (16-row groups: psum accumulation group and
        # fp8 moving operand are both capped at 512 output cols)
        mms = []
        for (dy, dx) in SHIFTS:
            oy, ox = dy - 1, dx - 1
            r0 = max(r_base, -oy)
            r1 = min(r_base + RH, H - oy)
            c0 = max(0, -ox)
            c1 = min(W, W - ox)
            k = dy * 3 + dx
            for q0 in range(r_base, r_base + RH, 16):
                s0 = max(q0, r0)
                s1 = min(min(q0 + 16, r_base + RH), r1)
                if s1 <= s0:
                    continue
                # tap (1,1) is emitted first and fully covers each 16-row
                # group: its MMs carry start=True (PSUM reset per word)
                st = (dy, dx) == (1, 1)
                for (w_sb, x3d) in terms:
                    mms.append((
                        p3[:, s0 - r_base:s1 - r_base, c0:c1],
                        w_sb[:, k, :],
                        x3d[:, s0 + oy:s1 + oy, c0 + ox:c1 + ox],
                        None, st and (w_sb, x3d) == terms[0]))
                for (w8, x8) in dr_terms:
                    mms.append((
                        p3[:, s0 - r_base:s1 - r_base, c0:c1],
                        w8[:, k],
                        x8[:, :, s0 + oy:s1 + oy, c0 + ox:c1 + ox],
                        mybir.MatmulPerfMode.DoubleRow, False))
        for i_mm, (out_ap, w_ap, x_ap, pm, st) in enumerate(mms):
            nc.tensor.matmul(out_ap, w_ap, x_ap,
                             start=st, stop=(i_mm == len(mms) - 1),
                             perf_mode=pm)
        _drain(nc, y_img, pt, h, acc, drain_scale)


def _conv1_f32r(nc, psum_pool, y_img, w_sb, xpad, acc=None,
                drain_scale=1.0):
    """Layer-1 conv, single f32r term over a zero-padded [C,34,34] input.
    f32r matmuls need an even column count, so every tap runs full-width
    (16 rows x 32 cols) against the padded buffer: 18 MMs x 512 cols at
    1 cycle/col."""
    pstride = xpad.ap[0][0]
    base_off = xpad.offset
    n_mm = 9 * (RH // 16) * NHALF // NHALF
    for h in range(NHALF):
        pt = psum_pool.tile([C, PSUM_COLS], F32, tag="pt")
        p3 = pt.rearrange("c (r w) -> c r w", r=RH)
        i_mm = 0
        n_tile = 9 * (RH // 16)
        for g in range(RH // 16):
            r0 = h * RH + g * 16
            for k in range(9):
                dy, dx = k // 3, k % 3
                mv = AP(xpad.tensor, base_off + (r0 + dy) * WP + dx,
                        [[pstride, C], [WP, 16], [1, W]])
                nc.tensor.matmul(
                    p3[:, g * 16:(g + 1) * 16, :], w_sb[:, k, :], mv,
                    start=(k == 0), stop=(i_mm == n_tile - 1))
                i_mm += 1
        _drain(nc, y_img, pt, h, acc, drain_scale)


def _drain(nc, y_img, pt, h, acc, drain_scale, y_scale=None):
    """Drain PSUM -> y (Act Copy, scale=drain_scale; accum -> sum slot) and
    accumulate sum(y_true^2) via Square with scale 1/y_scale, where y_scale
    is the TOTAL 4^t factor of the stored y (drain_scale for conv1, where
    the scale enters at the drain; 4^t for conv2/3, where it rides in the
    fp8 moving operand)."""
    if y_scale is None:
        y_scale = drain_scale
    y_half = y_img[:, h * PSUM_COLS:(h + 1) * PSUM_COLS]
    if acc is None:
        nc.scalar.activation(y_half, pt, AF.Copy, scale=drain_scale)
        return
    sum_slot, sq_slot, scratch_pool, slot = acc
    nc.scalar.activation(y_half, pt, AF.Copy, scale=drain_scale,
                         accum_out=sum_slot[:, slot + h:slot + h + 1])
    ysub = y_half.rearrange("c (p s) -> c p s", s=SQS)[:, :, 0]
    scr = scratch_pool.tile([C, PSUM_COLS // SQS], FP16, tag="sqscr")
    nc.scalar.activation(scr, ysub, AF.Square, scale=1.0 / y_scale,
                         accum_out=sq_slot[:, slot + h:slot + h + 1])


def _conv_dr(nc, psum_pool, y_img, wpk, x8pad, acc=None, drain_scale=1.0,
             y_scale=1.0):
    """Layer 2/3 conv for one image: tap-paired e4m3 DR matmuls reading the
    zero-padded spike buffer, in 16-row groups (fp8 moving operand is capped
    at 1024 elements = 512 output cols).  The moving operand's 4^t/WS scale
    makes the PSUM result 4^t * y; drain scale stays 1."""
    pstride = x8pad.ap[0][0]
    base_off = x8pad.offset
    n_mm = len(PAIRS) * (RH // 16)
    for h in range(NHALF):
        pt = psum_pool.tile([C, PSUM_COLS], F32, tag="pt")
        p3 = pt.rearrange("c (r w) -> c r w", r=RH)
        i_mm = 0
        for g in range(RH // 16):
            r0 = h * RH + g * 16
            for m, ((ka, _), (kb, _)) in enumerate(PAIRS):
                fa, fb = _flat(ka), _flat(kb)
                mv = AP(x8pad.tensor, base_off + r0 * WP + fa,
                        [[pstride, C], [fb - fa, 2], [WP, 16], [1, W]])
                nc.tensor.matmul(
                    p3[:, g * 16:(g + 1) * 16, :], wpk[:, m], mv,
                    start=(m == 0), stop=(i_mm == n_mm - 1),
                    perf_mode=mybir.MatmulPerfMode.DoubleRow,
                )
                i_mm += 1
        _drain(nc, y_img, pt, h, acc, drain_scale, y_scale)


def _layer_stats(nc, sb1, stats_buf, pre, eps_t, wvec, pow4, cc_in, cc_out,
                 lname):
    """Weighted stats reduce + AllGather (or AllReduce) + u-space
    coefficients, expanded to per-timestep 4^t tiles.

    Returns (dp4, thp4, negthp4): [C, T] tiles, column t = 4^t * coeff."""
    sums, sqs = stats_buf
    cc = sb1.tile([C, 2], F32, tag=f"cc{lname}")
    tmp = sb1.tile([C, SLOTS], F32, tag=f"ccw{lname}")
    nc.vector.tensor_tensor(tmp, sums, wvec, op=ALU.mult)
    nc.vector.tensor_reduce(cc[:, 0:1], tmp,
                            axis=mybir.AxisListType.X, op=ALU.add)
    nc.vector.tensor_reduce(cc[:, 1:2], sqs,
                            axis=mybir.AxisListType.X, op=ALU.add)
    nc.sync.dma_start(out=cc_in[:, :], in_=cc)
    ccs = sb1.tile([C, 2], F32, tag=f"ccs{lname}")
    if SIM1 or NOAR:
        nc.sync.dma_start(out=ccs, in_=cc_in[:, :])
    elif AG:
        nc.gpsimd.collective_compute(
            "AllGather", ALU.bypass,
            replica_groups=[list(range(NCORES))],
            ins=[cc_in[:, :]], outs=[cc_out[:, :]],
        )
        ccr = sb1.tile([C, 2, NCORES], F32, tag=f"ccr{lname}")
        nc.sync.dma_start(
            out=ccr, in_=cc_out[:, :].rearrange("(g c) k -> c k g", g=NCORES))
        nc.vector.tensor_reduce(ccs.unsqueeze(2), ccr,
                                axis=mybir.AxisListType.X, op=ALU.add)
    else:
        nc.gpsimd.collective_compute(
            "AllReduce", ALU.add,
            replica_groups=[list(range(NCORES))],
            ins=[cc_in[:, :]], outs=[cc_out[:C, :]],
        )
        nc.sync.dma_start(out=ccs, in_=cc_out[:C, :])
    inv_n = 1.0 / ((1 if SIM1 else NCORES) * NIMG * HW)
    # short critical-path algebra: th' = 0.5*sd/bnw, d' = bnb*sd/bnw - mean
    rbw, nrbw, bbrb = pre
    mean = sb1.tile([C, 1], F32, tag=f"mean{lname}")
    nc.vector.tensor_scalar(mean, ccs[:, 0:1], inv_n, None, op0=ALU.mult)
    msq = sb1.tile([C, 1], F32, tag=f"msq{lname}")
    nc.vector.tensor_tensor(msq, mean, mean, op=ALU.mult)
    var = sb1.tile([C, 1], F32, tag=f"var{lname}")
    nc.vector.scalar_tensor_tensor(var, ccs[:, 1:2], inv_n * SQS, msq,
                                   op0=ALU.mult, op1=ALU.subtract)
    sd = sb1.tile([C, 1], F32, tag=f"sd{lname}")
    nc.scalar.activation(sd, var, AF.Sqrt, bias=eps_t, scale=1.0)
    thp = sb1.tile([C, 1], F32, tag=f"thp{lname}")
    nc.vector.tensor_tensor(thp, sd, rbw, op=ALU.mult)
    dp = sb1.tile([C, 1], F32, tag=f"dp{lname}")
    nc.vector.scalar_tensor_tensor(dp, sd, bbrb, mean,
                                   op0=ALU.mult, op1=ALU.subtract)
    # expand to per-timestep 4^t columns (one op each via pow4 broadcast)
    thp4 = sb1.tile([C, T], F32, tag=f"thp4{lname}")
    nc.vector.tensor_tensor(thp4, thp.broadcast_to([C, T]), pow4,
                            op=ALU.mult)
    dp4 = sb1.tile([C, T], F32, tag=f"dp4{lname}")
    nc.vector.tensor_tensor(dp4, dp.broadcast_to([C, T]), pow4, op=ALU.mult)
    negthp4 = sb1.tile([C, T], F32, tag=f"negthp4{lname}")
    nc.vector.tensor_scalar(negthp4, thp4, -1.0, None, op0=ALU.mult)
    return dp4, thp4, negthp4


def build():
    nc = bacc.Bacc("TRN2", target_bir_lowering=False, debug=False,
                   num_devices=1 if SIM1 else NCORES)

    # --- I/O ---
    x_dt = F32R if L1_MODE == "f32r" else FP16
    x_cols = HWP if L1_MODE == "f32r" else HW
    xhi_d = nc.dram_tensor("xhi", [T, BL, C, x_cols], x_dt,
                           kind="ExternalInput")
    xlo_d = xc8_d = w1c8_d = None
    if L1_MODE == "dr":
        xc8_d = nc.dram_tensor("xc8", [T, BL, C, 2, HW], FP8E5,
                               kind="ExternalInput")
        w1c8_d = nc.dram_tensor("w1c8", [C, 9, 2, C], FP8E5,
                                kind="ExternalInput")
    elif TERMS_L1 >= 2:
        xlo_d = nc.dram_tensor("xlo", [T, BL, C, HW], FP16,
                               kind="ExternalInput")
    w_d = {(1, "hi"): nc.dram_tensor("w1hi", [C, 9, C], x_dt,
                                     kind="ExternalInput")}
    if TERMS_L1 >= 3:
        w_d[(1, "lo")] = nc.dram_tensor("w1lo", [C, 9, C], FP16,
                                        kind="ExternalInput")
    wpk_d = {}
    for l in (2, 3):
        wpk_d[l] = nc.dram_tensor(f"wpk{l}", [C, 9, 2, C], FP8E4,
                                  kind="ExternalInput")
    bn_d = {}
    for l in (1, 2, 3):
        bn_d[(l, "w")] = nc.dram_tensor(f"bnw{l}", [C, 1], F32,
                                        kind="ExternalInput")
        bn_d[(l, "b")] = nc.dram_tensor(f"bnb{l}", [C, 1], F32,
                                        kind="ExternalInput")
    fcw_d = nc.dram_tensor("fcw", [C, 10], F32, kind="ExternalInput")
    fcb_d = nc.dram_tensor("fcb", [1, 10], F32, kind="ExternalInput")
    wvec_d = nc.dram_tensor("wvec", [C, SLOTS], F32, kind="ExternalInput")
    pow4_d = nc.dram_tensor("pow4", [C, T], F32, kind="ExternalInput")
    out_d = nc.dram_tensor("out", [1, BL * 10], F32, kind="ExternalOutput")

    # --- internal DRAM (collective buffers only) ---
    cc_bufs = {}
    for l in (1, 2, 3):
        cc_bufs[l] = (
            nc.dram_tensor(f"cc_in{l}", [C, 2], F32),
            nc.dram_tensor(f"cc_out{l}", [C * (NCORES if AG else 1), 2], F32,
                           addr_space="Shared"),
        )

    with ExitStack() as ctx:
        tc = ctx.enter_context(tile.TileContext(nc))
        sb1 = ctx.enter_context(tc.tile_pool(name="sb1", bufs=1))
        xpool = ctx.enter_context(tc.tile_pool(name="xpool", bufs=4))
        gate_pool = ctx.enter_context(tc.tile_pool(name="gate", bufs=3))
        stage_pool = ctx.enter_context(tc.tile_pool(name="stage", bufs=3))
        mem_pool = ctx.enter_context(tc.tile_pool(name="mem", bufs=1))
        psum_pool = ctx.enter_context(
            tc.tile_pool(name="psum", bufs=(8 * 512) // PSUM_COLS,
                         space="PSUM"))

        # --- load layer-1 weights only; everything else streams during
        # the conv1 window ---
        w_sb = {}
        for key in ((1, "hi"),) + (((1, "lo"),) if (1, "lo") in w_d else ()):
            w_sb[key] = sb1.tile([C, 9, C], x_dt,
                                 name=f"w{key[0]}{key[1]}",
                                 tag=f"w{key[0]}{key[1]}")
            nc.sync.dma_start(out=w_sb[key], in_=w_d[key][:, :, :])
        w1c8_sb = None
        if L1_MODE == "dr":
            w1c8_sb = sb1.tile([C, 9, 2, C], FP8E5, name="w1c8", tag="w1c8")
            nc.sync.dma_start(out=w1c8_sb, in_=w1c8_d[:, :, :, :])

        # Warm-up matmuls: touch each weight tensor once with a fused
        # (self-loading) matmul so the weight-DMA waits attach to a Matmult.
        def warmup_mm(w2d):
            pt = psum_pool.tile([C, PSUM_COLS], F32, tag="pt")
            nc.tensor.matmul(pt[:, 0:8], w2d, w2d[:, 0:8],
                             start=True, stop=True)

        warmup_mm(w_sb[(1, "hi")][:, 4, :])
        if L1_MODE == "dr":
            warmup_mm(w1c8_sb[:, 4, 0, :])

        # persistent SBUF activations: y in f32 (per-image scaled by 4^t)
        ybuf = sb1.tile([C, NIMG, HW], F32, name="ybuf")
        # padded fp8 spike buffers: pad ring stays zero (conv zero-padding);
        # 8 rotating persistent slots (WAR tracked by the tile framework)
        x8bufs = []
        for i in range(8):
            xb = sb1.tile([C, HWP], FP8E4, name=f"x8b{i}", tag=f"x8b{i}")
            nc.vector.memset(xb, 0.0)
            x8bufs.append(xb)

        # L3 pooled spike counts per (t, b)
        poolbuf = sb1.tile([C, BL, T], F32)

        stats = {}
        for l in (1, 2, 3):
            ysum = sb1.tile([C, SLOTS], F32, name=f"ysum{l}",
                            tag=f"sum{l}")
            ysq = sb1.tile([C, SLOTS], F32, name=f"ysq{l}",
                           tag=f"sq{l}")
            stats[l] = (ysum, ysq)
        scratch_pool = ctx.enter_context(tc.tile_pool(name="sqscr", bufs=2))

        # =============== layer 1 conv (standalone window) ===============
        for t in range(T):
            for b in range(BL):
                img = t * BL + b
                xhi = xpool.tile([C, x_cols], x_dt, tag="xin")
                nc.sync.dma_start(out=xhi, in_=xhi_d[t, b, :, :])
                if L1_MODE == "f32r":
                    _conv1_f32r(nc, psum_pool, ybuf[:, img],
                                w_sb[(1, "hi")], xhi,
                                acc=(stats[1][0], stats[1][1], scratch_pool,
                                     NHALF * img),
                                drain_scale=float(4.0 ** t))
                    continue
                xhi3 = xhi.rearrange("c (h w) -> c h w", h=H)
                terms = [(w_sb[(1, "hi")], xhi3)]
                dr_terms = []
                if L1_MODE == "dr":
                    x8 = xpool.tile([C, 2, HW], FP8E5, tag="x8")
                    nc.sync.dma_start(out=x8, in_=xc8_d[t, b])
                    x84 = x8.rearrange("c two (h w) -> c two h w", h=H)
                    dr_terms.append((w1c8_sb, x84))
                else:
                    if TERMS_L1 >= 3:
                        terms.append((w_sb[(1, "lo")], xhi3))
                    if TERMS_L1 >= 2:
                        xlo = xpool.tile([C, HW], FP16, tag="xin")
                        nc.sync.dma_start(out=xlo, in_=xlo_d[t, b, :, :])
                        xlo3 = xlo.rearrange("c (h w) -> c h w", h=H)
                        terms.append((w_sb[(1, "hi")], xlo3))
                _conv_image(nc, psum_pool, ybuf[:, img], terms, dr_terms,
                            acc=(stats[1][0], stats[1][1], scratch_pool,
                                 NHALF * img),
                            drain_scale=float(4.0 ** t))

        # deferred constants: stream in under the conv1 window
        wpk_sb = {}
        for l in (2, 3):
            wpk_sb[l] = sb1.tile([C, 9, 2, C], FP8E4, name=f"wpk{l}",
                                 tag=f"wpk{l}")
            nc.sync.dma_start(out=wpk_sb[l], in_=wpk_d[l][:, :, :, :])
        bn_sb = {}
        for key, dt_ in bn_d.items():
            bn_sb[key] = sb1.tile([C, 1], F32, name=f"bn{key[1]}{key[0]}",
                                  tag=f"bn{key[1]}{key[0]}")
            nc.sync.dma_start(out=bn_sb[key], in_=dt_[:, :])
        fcw_sb = sb1.tile([C, 10], F32)
        nc.sync.dma_start(out=fcw_sb, in_=fcw_d[:, :])
        fcb_sb = sb1.tile([1, 10], F32)
        nc.sync.dma_start(out=fcb_sb, in_=fcb_d[:, :])
        wvec = sb1.tile([C, SLOTS], F32)
        nc.sync.dma_start(out=wvec, in_=wvec_d[:, :])
        pow4 = sb1.tile([C, T], F32)
        nc.sync.dma_start(out=pow4, in_=pow4_d[:, :])
        eps_t = sb1.tile([C, 1], F32)
        nc.vector.memset(eps_t, BN_EPS)
        # off-critical-path per-layer constants: 0.5/bnw, -0.5/bnw, bnb/bnw
        pre = {}
        for l in (1, 2, 3):
            rb = sb1.tile([C, 1], F32, tag=f"rbw{l}")
            nc.vector.reciprocal(out=rb, in_=bn_sb[(l, "w")])
            rbw = sb1.tile([C, 1], F32, tag=f"rbwth{l}")
            nc.vector.tensor_scalar(rbw, rb, THRESH, None, op0=ALU.mult)
            nrbw = sb1.tile([C, 1], F32, tag=f"nrbw{l}")
            nc.vector.tensor_scalar(nrbw, rb, -THRESH, None, op0=ALU.mult)
            bbrb = sb1.tile([C, 1], F32, tag=f"bbrb{l}")
            nc.vector.tensor_tensor(bbrb, bn_sb[(l, "b")], rb, op=ALU.mult)
            pre[l] = (rbw, nrbw, bbrb)
        # fp8 DR warm-ups for the deferred weights (moving data: zeroed x8)
        x8v0 = x8bufs[0].rearrange("c (h w) -> c h w", h=HP)
        for l in (2, 3):
            pt = psum_pool.tile([C, PSUM_COLS], F32, tag="pt")
            nc.tensor.matmul(pt[:, 0:1], wpk_sb[l][:, 4], x8v0[:, 0:2, 0:1],
                             start=True, stop=True,
                             perf_mode=mybir.MatmulPerfMode.DoubleRow)

        dp4_1, thp4_1, _ = _layer_stats(nc, sb1, stats[1], pre[1], eps_t,
                                        wvec, pow4, *cc_bufs[1], "l1")

        # ====== windows 2&3: LIF_l (DVE) interleaved with conv_{l+1} ======
        def lif_conv_window(dp4, thp4, wpk, stats_next):
            """Per image: 4^t-space LIF (r, U', pack) then the DR conv of
            the next layer reading the packed padded spikes."""
            u = mem_pool.tile([C, BL * HW], F32, tag="mem")
            u4 = u.rearrange("c (b p) -> c b p", b=BL)
            for t in range(T):
                # moving fp8 values must stay <= 2^6 (hw e4m3 treats biased
                # exponent 15 as inf/nan): cap the pack scale at 4^6/WS and
                # recover the remaining factor in the drain scale
                pk_scale = float(4.0 ** min(t, 6) / WS)
                ds = float(4.0 ** t / (4.0 ** min(t, 6)))
                for b in range(BL):
                    img = t * BL + b
                    yt = ybuf[:, img]
                    if t == 0:
                        nc.vector.tensor_scalar(u4[:, b], yt,
                                                dp4[:, 0:1], None,
                                                op0=ALU.add)
                    else:
                        # r: U <- U * [U <= thp4[t-1]]  (hard reset)
                        nc.vector.scalar_tensor_tensor(
                            u4[:, b], u4[:, b], thp4[:, t - 1:t], u4[:, b],
                            op0=ALU.is_le, op1=ALU.mult)
                        # U': U <- (y4 + dp4[t]) + U
                        nc.vector.scalar_tensor_tensor(
                            u4[:, b], yt, dp4[:, t:t + 1], u4[:, b],
                            op0=ALU.add, op1=ALU.add)
                    x8 = x8bufs[img % 8]
                    x8int = x8.rearrange("c (h w) -> c h w",
                                         h=HP)[:, 1:H + 1, 1:W + 1]
                    eng = nc.gpsimd if PACK_POOL else nc.vector
                    eng.tensor_scalar(
                        x8int, u4[:, b].rearrange("c (h w) -> c h w", h=H),
                        thp4[:, t:t + 1], pk_scale,
                        op0=ALU.is_gt, op1=ALU.mult)
                    _conv_dr(nc, psum_pool, ybuf[:, img], wpk, x8,
                             acc=(stats_next[0], stats_next[1],
                                  scratch_pool, NHALF * img),
                             drain_scale=ds, y_scale=float(4.0 ** t))

        lif_conv_window(dp4_1, thp4_1, wpk_sb[2], stats[2])
        dp4_2, thp4_2, _ = _layer_stats(nc, sb1, stats[2], pre[2], eps_t,
                                        wvec, pow4, *cc_bufs[2], "l2")

        lif_conv_window(dp4_2, thp4_2, wpk_sb[3], stats[3])
        dp4_3, thp4_3, negthp4_3 = _layer_stats(nc, sb1, stats[3], pre[3],
                                                eps_t, wvec, pow4,
                                                *cc_bufs[3], "l3")

        # =============== layer 3 LIF (4^t space) ===============
        # b 0/1: pure-DVE r-form (reset mask folded into one stt; no
        # cross-engine dependency in the recurrence — the Act Sign is a
        # pooling-only observer).  b 2/3: sign-form with gate+mult on
        # GpSimd (float-scalar ops only; stt is DVE-only in the ISA).
        u = mem_pool.tile([C, BL * HW], F32, tag="mem")
        u4 = u.rearrange("c (b p) -> c b p", b=BL)
        prev_sgn = [None] * BL
        for t in range(T):
            for b in range(BL):
                img = t * BL + b
                yt = ybuf[:, img]
                if t == 0:
                    nc.vector.tensor_scalar(u4[:, b], yt, dp4_3[:, 0:1],
                                            None, op0=ALU.add)
                else:
                    if b < 2:
                        nc.vector.scalar_tensor_tensor(
                            u4[:, b], u4[:, b], thp4_3[:, t - 1:t], u4[:, b],
                            op0=ALU.is_le, op1=ALU.mult)
                    else:
                        gate = gate_pool.tile([C, HW], FP16, tag="gate")
                        nc.gpsimd.tensor_scalar(gate, prev_sgn[b], -0.5, 0.5,
                                                op0=ALU.mult, op1=ALU.add)
                        nc.gpsimd.tensor_tensor(u4[:, b], u4[:, b], gate,
                                                op=ALU.mult)
                    nc.vector.scalar_tensor_tensor(
                        u4[:, b], yt, dp4_3[:, t:t + 1], u4[:, b],
                        op0=ALU.add, op1=ALU.add)
                st = stage_pool.tile([C, HW], FP16, tag="stage")
                nc.scalar.activation(st, u4[:, b], AF.Sign,
                                     bias=negthp4_3[:, t:t + 1], scale=1.0,
                                     accum_out=poolbuf[:, b, t:t + 1])
                prev_sgn[b] = st

        # =============== head: pooling + FC ===============
        feat = sb1.tile([C, BL], F32)
        nc.vector.tensor_reduce(feat.unsqueeze(2), poolbuf,
                                axis=mybir.AxisListType.X, op=ALU.add)
        nc.vector.tensor_scalar(feat, feat, 0.5, T * HW / 2.0,
                                op0=ALU.mult, op1=ALU.add)
        prod = sb1.tile([C, BL, 10], F32)
        nc.vector.tensor_tensor(
            prod, feat.unsqueeze(2).broadcast_to([C, BL, 10]),
            fcw_sb.unsqueeze(1).broadcast_to([C, BL, 10]), op=ALU.mult)
        red = sb1.tile([C, BL, 10], F32)
        nc.gpsimd.partition_all_reduce(red, prod, channels=C,
                                       reduce_op=bass_isa.ReduceOp.add)
        ofin = sb1.tile([1, BL, 10], F32)
        nc.vector.tensor_scalar(ofin, red[0:1], 1.0 / POOL_N, None,
                                op0=ALU.mult)
        nc.vector.tensor_tensor(
            ofin, ofin, fcb_sb.unsqueeze(1).broadcast_to([1, BL, 10]),
            op=ALU.add)
        nc.sync.dma_start(out=out_d[:, :],
                          in_=ofin.rearrange("c b k -> c (b k)"))

        if DBG:
            d_ybuf = nc.dram_tensor("d_ybuf", [C, NIMG * HW], F32,
                                    kind="ExternalOutput")
            nc.sync.dma_start(out=d_ybuf[:, :],
                              in_=ybuf.rearrange("c n p -> c (n p)"))
            d_pool = nc.dram_tensor("d_pool", [C, BL * T], F32,
                                    kind="ExternalOutput")
            nc.sync.dma_start(out=d_pool[:, :],
                              in_=poolbuf.rearrange("c b t -> c (b t)"))
            for l in (1, 2, 3):
                ds = nc.dram_tensor(f"d_sum{l}", [C, SLOTS], F32,
                                    kind="ExternalOutput")
                nc.sync.dma_start(out=ds[:, :], in_=stats[l][0])
                dq = nc.dram_tensor(f"d_sq{l}", [C, SLOTS], F32,
                                    kind="ExternalOutput")
                nc.sync.dma_start(out=dq[:, :], in_=stats[l][1])
            for i in range(8):
                dx8 = nc.dram_tensor(f"d_x8_{i}", [C, HWP], FP8E4,
                                     kind="ExternalOutput")
                nc.sync.dma_start(out=dx8[:, :], in_=x8bufs[i])
            for l, t4 in (("dp1", dp4_1), ("th1", thp4_1),
                          ("dp3", dp4_3), ("nth3", negthp4_3)):
                dt4 = nc.dram_tensor(f"d_{l}", [C, T], F32,
                                     kind="ExternalOutput")
                nc.sync.dma_start(out=dt4[:, :], in_=t4)

    # walrus rejects the standalone InstLdweights this pass splits out for
    # our multi-wait first-of-image matmuls; excess waits lower to
    # event-semaphore chains instead.
    nc.move_matmul_waits_to_ldweights = lambda: None
    nc.compile()
    return nc


_NC_CACHE = {}


def _get_nc():
    if "nc" not in _NC_CACHE:
        _NC_CACHE["nc"] = build()
    return _NC_CACHE["nc"]


def _hi_lo(a):
    hi = a.astype(np.float16)
    lo = (a - hi.astype(np.float32)).astype(np.float16)
    return hi, lo


def _e5m2(a, scale):
    return (np.asarray(a, np.float32) * scale).astype(ml_dtypes.float8_e5m2)


def _e4m3(a):
    return np.asarray(a, np.float32).astype(ml_dtypes.float8_e4m3fn)


def make_in_maps(inp, conv_ws, bns, fc_w, fc_b):
    """Build the 8 per-core input maps from full (numpy) model inputs."""
    common = {}
    for li, w in enumerate(conv_ws, start=1):
        wt = np.ascontiguousarray(
            w.transpose(1, 2, 3, 0).reshape(C, 9, C))  # [I, k, O]
        if li == 1:
            if L1_MODE == "f32r":
                common["w1hi"] = wt.astype(np.float32)
            else:
                hi, lo = _hi_lo(wt)
                common["w1hi"] = hi
            if L1_MODE == "dr":
                w1c8 = np.empty((C, 9, 2, C), dtype=ml_dtypes.float8_e5m2)
                w1c8[:, :, 0, :] = _e5m2(lo.astype(np.float32), DR_S)
                w1c8[:, :, 1, :] = _e5m2(hi.astype(np.float32), 1.0 / DR_S)
                common["w1c8"] = w1c8
            if TERMS_L1 >= 3:
                common["w1lo"] = lo
        else:
            ws_hi = _e4m3(wt * WS)
            ws_lo = _e4m3(wt * WS - ws_hi.astype(np.float32))
            terms = (ws_hi, ws_lo)
            wpk = np.empty((C, 9, 2, C), dtype=ml_dtypes.float8_e4m3fn)
            for m, ((ka, ta), (kb, tb)) in enumerate(PAIRS):
                wpk[:, m, 0, :] = terms[ta][:, ka, :]
                wpk[:, m, 1, :] = terms[tb][:, kb, :]
            common[f"wpk{li}"] = wpk
        common[f"bnw{li}"] = np.ascontiguousarray(
            bns[li - 1][0].reshape(C, 1))
        common[f"bnb{li}"] = np.ascontiguousarray(
            bns[li - 1][1].reshape(C, 1))
    common["fcw"] = np.ascontiguousarray(fc_w.T)          # [C, 10]
    common["fcb"] = np.ascontiguousarray(fc_b.reshape(1, 10))
    wvec = np.zeros((1, SLOTS), np.float32)
    for t in range(T):
        for b in range(BL):
            img = t * BL + b
            for h in range(NHALF):
                wvec[0, NHALF * img + h] = 4.0 ** (-t)
    common["wvec"] = np.ascontiguousarray(np.broadcast_to(wvec, (C, SLOTS)))
    pow4 = np.asarray([[4.0 ** t for t in range(T)]], np.float32)
    common["pow4"] = np.ascontiguousarray(np.broadcast_to(pow4, (C, T)))

    in_maps = []
    for cid in range(NCORES):
        xc = np.ascontiguousarray(
            inp[:, cid * BL:(cid + 1) * BL].reshape(T, BL, C, HW))
        m = dict(common)
        if L1_MODE == "f32r":
            xp = np.zeros((T, BL, C, HP, WP), np.float32)
            xp[:, :, :, 1:H + 1, 1:W + 1] = xc.reshape(T, BL, C, H, W)
            m["xhi"] = xp.reshape(T, BL, C, HWP)
        else:
            xhi, xlo = _hi_lo(xc)
            m["xhi"] = xhi
        if L1_MODE == "dr":
            xc8 = np.empty((T, BL, C, 2, HW), dtype=ml_dtypes.float8_e5m2)
            xc8[:, :, :, 0, :] = _e5m2(xc, 1.0 / DR_S)
            xc8[:, :, :, 1, :] = _e5m2(xlo.astype(np.float32), DR_S)
            m["xc8"] = xc8
        elif TERMS_L1 >= 2:
            m["xlo"] = xlo
        in_maps.append(m)
    return in_maps


def kernel(inp, conv_w1, conv_w2, conv_w3, bn_w1, bn_b1, bn_w2, bn_b2,
           bn_w3, bn_b3, fc_w, fc_b):
    inp = np.asarray(inp, dtype=np.float32)
    ws = [np.asarray(w, dtype=np.float32) for w in (conv_w1, conv_w2, conv_w3)]
    bns = [(np.asarray(bn_w1, np.float32), np.asarray(bn_b1, np.float32)),
           (np.asarray(bn_w2, np.float32), np.asarray(bn_b2, np.float32)),
           (np.asarray(bn_w3, np.float32), np.asarray(bn_b3, np.float32))]
    fc_w = np.asarray(fc_w, np.float32)
    fc_b = np.asarray(fc_b, np.float32)

    nc = _get_nc()
    in_maps = make_in_maps(inp, ws, bns, fc_w, fc_b)
    res = run_bass_kernel_spmd(nc, in_maps, core_ids=list(range(NCORES)))
    out = np.concatenate(
        [r["out"].reshape(BL, 10) for r in res.results], axis=0)
    return out.astype(np.float32)


# revision 41
# speedup vs baseline: 13.9063x; 4.0991x over previous
"""Trainium2 Bass kernel for nn_Classifier_64587718197982 (spiking CNN).

Network (per reference):
  3x [conv3x3(C=128, pad=1, no bias) -> TDBN (batchnorm over T,B,H,W) -> LIF]
  -> mean over (H,W) -> mean over T -> FC(128->10)

Sharding: data-parallel over batch B=32 across 8 NeuronCores (4 images/core).
TDBN stats become a tiny per-layer AllGather + local sum.

v2 design (on top of the v1 SBUF-resident pipeline):
  - Layers 2/3 convs run entirely in fp8-e4m3 DoubleRow matmuls (0.5
    cycles/output column).  Spikes are exact in fp8; weights use a
    two-term hi+lo e4m3 split (~2^-8 relative weight error).  The two DR
    contraction halves carry TWO DIFFERENT (tap, term) pairs: spikes live
    in a zero-PADDED [C,34,34] fp8 buffer so every tap is a full-width
    matmul (pad ring = conv zero padding), and the moving AP's half-axis
    stride is the flat offset between the two taps.  9 DR matmuls of 512
    cols per psum half-image -> 2304 PE cycles/image (4x fewer than fp16).
  - LIF runs in 4^t-scaled membrane space U_t = 4^t * u_t, which absorbs
    the 0.25 decay into per-timestep constants: 3 full-size ops per step:
      r:    U <- U * [U <= thp*4^(t-1)]      (one scalar_tensor_tensor)
      U':   U <- (dp*4^t + y4) + U           (one scalar_tensor_tensor)
      pack: x8 <- [U > thp*4^t] * (4^t/64)   (one tensor_scalar, fp8 out)
    The 4^t scale on y comes for free: conv1 drains scale by 4^t (Act
    scale), conv2/3 moving spikes carry 4^t/64 while weights carry x64.
  - Stats: drain accum gives per-image-slot sums of 4^t*y; a host-staged
    4^-t weight vector unscales them at reduce time.  The Square pass
    reads y4 with Act scale 4^-t (squared internally -> unscaled sum(y^2))
    and subsamples columns by K_SQS (default 2).
  - Collective: AllGather of [C,2] + local sum (cheaper than AllReduce).
  - Tail (LIF3) keeps the v1 sign-domain form (gate = 0.5 - 0.5*sgn in
    4^t space) with the DVE/GpSimd split, Sign+accum pooling on Act.
"""
import numpy as np
import ml_dtypes
from contextlib import ExitStack

import concourse.bass as bass
import concourse.mybir as mybir
import concourse.tile as tile
from concourse.ap import AP
from concourse import bass_isa
from concourse import bacc
from concourse.bass_utils import run_bass_kernel_spmd

F32 = mybir.dt.float32
F32R = mybir.dt.float32r
FP16 = mybir.dt.float16
FP8E5 = mybir.dt.float8e5
FP8E4 = mybir.dt.float8e4
AF = mybir.ActivationFunctionType
ALU = mybir.AluOpType

T, B, C, H, W = 8, 32, 128, 32, 32
NCORES = 8
BL = B // NCORES          # images per core per timestep
HW = H * W                # 1024
NIMG = T * BL             # 32 images per core
DECAY = 0.25
THRESH = 0.5
BN_EPS = 1e-5
POOL_N = float(T * HW)    # pooling divisor 8192
HP, WP = H + 2, W + 2     # padded spike image
HWP = HP * WP             # 1156
WS = 64.0                 # weight scale for e4m3 two-term split
import os as _os
SIM1 = _os.environ.get("K_SIM1", "") == "1"  # single-core sim (no collectives)
NOAR = _os.environ.get("K_NOAR", "") == "1"  # timing probe: skip collective (wrong results)
AG = _os.environ.get("K_AG", "1") == "1"     # AllGather+local sum vs AllReduce
SQS = int(_os.environ.get("K_SQS", "1"))     # square-pass column stride
PACK_POOL = _os.environ.get("K_PACK_POOL", "1") == "1"  # pack op on gpsimd
# how many of the 4 per-t images run the LIF reset (r) op on gpsimd
RPOOL = int(_os.environ.get("K_RPOOL", "1"))
DBG = _os.environ.get("K_DBG", "") == "1"   # dump intermediates to DRAM

# layer-1 term scheme: "f32r" = single float32r term (1 cyc/col at
# free>=256, ~2x more accurate than fp16); "dr" = fp16 main + e5m2
# DoubleRow corrections; integers = fp16 multi-term schemes (1/2/3)
L1_MODE = _os.environ.get("K_L1", "f32r")
TERMS_L1 = 0 if L1_MODE in ("dr", "f32r") else int(L1_MODE)
# e5m2 correction-term balance scale (wlo*S paired with x/S; whi/S with xlo*S)
DR_S = 64.0
# psum tile columns (1024 = full image, 2 banks per tile)
PSUM_COLS = int(_os.environ.get("K_PSUM_COLS", "1024"))
NHALF = HW // PSUM_COLS
RH = H // NHALF
SLOTS = NIMG * NHALF

# conv shift order: (1,1) first so the start=True matmul covers the full tile
SHIFTS = [(1, 1)] + [(dy, dx) for dy in range(3) for dx in range(3)
                     if not (dy == 1 and dx == 1)]

# tap-pair table for the DR convs of layers 2/3: each matmul's two
# contraction halves are (tap, term) pairs; term 0 = hi = e4m3(w*WS),
# term 1 = lo = e4m3(w*WS - hi).  flat(k) = (k//3)*WP + k%3; the moving
# AP half-axis stride is flat(kb) - flat(ka) (all pairs chosen positive).
PAIRS = [((0, 0), (2, 0)), ((3, 0), (5, 0)), ((6, 0), (8, 0)),
         ((0, 1), (2, 1)), ((3, 1), (5, 1)), ((6, 1), (8, 1)),
         ((1, 0), (7, 0)), ((1, 1), (4, 0)), ((4, 1), (7, 1))]


def _flat(k):
    return (k // 3) * WP + (k % 3)


def _conv_image(nc, psum_pool, y_img, terms, dr_terms=(), acc=None,
                drain_scale=1.0):
    """Layer-1 conv for one image (fp16 main + e5m2 DR corrections).
    Drains PSUM (Act, scale=drain_scale); accum_out gives sum(scale*y);
    a second Act pass (Square, scale=1/drain_scale) accumulates sum(y^2)
    over a 1/SQS column subsample."""
    for h in range(NHALF):
        r_base = h * RH
        pt = psum_pool.tile([C, RH * W], F32, tag="pt")
        p3 = pt.rearrange("c (r w) -> c r w", r=RH)
        # collect all matmuls (16-row groups: psum accumulation group and
        # fp8 moving operand are both capped at 512 output cols)
        mms = []
        for (dy, dx) in SHIFTS:
            oy, ox = dy - 1, dx - 1
            r0 = max(r_base, -oy)
            r1 = min(r_base + RH, H - oy)
            c0 = max(0, -ox)
            c1 = min(W, W - ox)
            k = dy * 3 + dx
            for q0 in range(r_base, r_base + RH, 16):
                s0 = max(q0, r0)
                s1 = min(min(q0 + 16, r_base + RH), r1)
                if s1 <= s0:
                    continue
                # tap (1,1) is emitted first and fully covers each 16-row
                # group: its MMs carry start=True (PSUM reset per word)
                st = (dy, dx) == (1, 1)
                for (w_sb, x3d) in terms:
                    mms.append((
                        p3[:, s0 - r_base:s1 - r_base, c0:c1],
                        w_sb[:, k, :],
                        x3d[:, s0 + oy:s1 + oy, c0 + ox:c1 + ox],
                        None, st and (w_sb, x3d) == terms[0]))
                for (w8, x8) in dr_terms:
                    mms.append((
                        p3[:, s0 - r_base:s1 - r_base, c0:c1],
                        w8[:, k],
                        x8[:, :, s0 + oy:s1 + oy, c0 + ox:c1 + ox],
                        mybir.MatmulPerfMode.DoubleRow, False))
        for i_mm, (out_ap, w_ap, x_ap, pm, st) in enumerate(mms):
            nc.tensor.matmul(out_ap, w_ap, x_ap,
                             start=st, stop=(i_mm == len(mms) - 1),
                             perf_mode=pm)
        _drain(nc, y_img, pt, h, acc, drain_scale)


def _conv1_f32r(nc, psum_pool, y_img, w_sb, xpad, acc=None,
                drain_scale=1.0):
    """Layer-1 conv, single f32r term over a zero-padded [C,34,34] input.
    f32r matmuls need an even column count, so every tap runs full-width
    (16 rows x 32 cols) against the padded buffer: 18 MMs x 512 cols at
    1 cycle/col."""
    pstride = xpad.ap[0][0]
    base_off = xpad.offset
    n_mm = 9 * (RH // 16) * NHALF // NHALF
    for h in range(NHALF):
        pt = psum_pool.tile([C, PSUM_COLS], F32, tag="pt")
        p3 = pt.rearrange("c (r w) -> c r w", r=RH)
        i_mm = 0
        n_tile = 9 * (RH // 16)
        for g in range(RH // 16):
            r0 = h * RH + g * 16
            for k in range(9):
                dy, dx = k // 3, k % 3
                mv = AP(xpad.tensor, base_off + (r0 + dy) * WP + dx,
                        [[pstride, C], [WP, 16], [1, W]])
                nc.tensor.matmul(
                    p3[:, g * 16:(g + 1) * 16, :], w_sb[:, k, :], mv,
                    start=(k == 0), stop=(i_mm == n_tile - 1))
                i_mm += 1
        _drain(nc, y_img, pt, h, acc, drain_scale)


def _drain(nc, y_img, pt, h, acc, drain_scale, y_scale=None):
    """Drain PSUM -> y (Act Copy, scale=drain_scale; accum -> sum slot) and
    accumulate sum(y_true^2) via Square with scale 1/y_scale, where y_scale
    is the TOTAL 4^t factor of the stored y (drain_scale for conv1, where
    the scale enters at the drain; 4^t for conv2/3, where it rides in the
    fp8 moving operand)."""
    if y_scale is None:
        y_scale = drain_scale
    y_half = y_img[:, h * PSUM_COLS:(h + 1) * PSUM_COLS]
    if acc is None:
        nc.scalar.activation(y_half, pt, AF.Copy, scale=drain_scale)
        return
    sum_slot, sq_slot, scratch_pool, slot = acc
    nc.scalar.activation(y_half, pt, AF.Copy, scale=drain_scale,
                         accum_out=sum_slot[:, slot + h:slot + h + 1])
    ysub = y_half.rearrange("c (p s) -> c p s", s=SQS)[:, :, 0]
    scr = scratch_pool.tile([C, PSUM_COLS // SQS], FP16, tag="sqscr")
    nc.scalar.activation(scr, ysub, AF.Square, scale=1.0 / y_scale,
                         accum_out=sq_slot[:, slot + h:slot + h + 1])


def _conv_dr(nc, psum_pool, y_img, wpk, x8pad, acc=None, drain_scale=1.0,
             y_scale=1.0):
    """Layer 2/3 conv for one image: tap-paired e4m3 DR matmuls reading the
    zero-padded spike buffer, in 16-row groups (fp8 moving operand is capped
    at 1024 elements = 512 output cols).  The moving operand's 4^t/WS scale
    makes the PSUM result 4^t * y; drain scale stays 1."""
    pstride = x8pad.ap[0][0]
    base_off = x8pad.offset
    n_mm = len(PAIRS) * (RH // 16)
    for h in range(NHALF):
        pt = psum_pool.tile([C, PSUM_COLS], F32, tag="pt")
        p3 = pt.rearrange("c (r w) -> c r w", r=RH)
        i_mm = 0
        for g in range(RH // 16):
            r0 = h * RH + g * 16
            for m, ((ka, _), (kb, _)) in enumerate(PAIRS):
                fa, fb = _flat(ka), _flat(kb)
                mv = AP(x8pad.tensor, base_off + r0 * WP + fa,
                        [[pstride, C], [fb - fa, 2], [WP, 16], [1, W]])
                nc.tensor.matmul(
                    p3[:, g * 16:(g + 1) * 16, :], wpk[:, m], mv,
                    start=(m == 0), stop=(i_mm == n_mm - 1),
                    perf_mode=mybir.MatmulPerfMode.DoubleRow,
                )
                i_mm += 1
        _drain(nc, y_img, pt, h, acc, drain_scale, y_scale)


def _layer_stats(nc, sb1, stats_buf, pre, eps_t, wvec, pow4, cc_in, cc_out,
                 lname):
    """Weighted stats reduce + AllGather (or AllReduce) + u-space
    coefficients, expanded to per-timestep 4^t tiles.

    Returns (dp4, thp4, negthp4): [C, T] tiles, column t = 4^t * coeff."""
    sums, sqs = stats_buf
    cc = sb1.tile([C, 2], F32, tag=f"cc{lname}")
    tmp = sb1.tile([C, SLOTS], F32, tag=f"ccw{lname}")
    nc.vector.tensor_tensor(tmp, sums, wvec, op=ALU.mult)
    nc.vector.tensor_reduce(cc[:, 0:1], tmp,
                            axis=mybir.AxisListType.X, op=ALU.add)
    nc.vector.tensor_reduce(cc[:, 1:2], sqs,
                            axis=mybir.AxisListType.X, op=ALU.add)
    nc.sync.dma_start(out=cc_in[:, :], in_=cc)
    ccs = sb1.tile([C, 2], F32, tag=f"ccs{lname}")
    if SIM1 or NOAR:
        nc.sync.dma_start(out=ccs, in_=cc_in[:, :])
    elif AG:
        nc.gpsimd.collective_compute(
            "AllGather", ALU.bypass,
            replica_groups=[list(range(NCORES))],
            ins=[cc_in[:, :]], outs=[cc_out[:, :]],
        )
        ccr = sb1.tile([C, 2, NCORES], F32, tag=f"ccr{lname}")
        nc.sync.dma_start(
            out=ccr, in_=cc_out[:, :].rearrange("(g c) k -> c k g", g=NCORES))
        nc.vector.tensor_reduce(ccs.unsqueeze(2), ccr,
                                axis=mybir.AxisListType.X, op=ALU.add)
    else:
        nc.gpsimd.collective_compute(
            "AllReduce", ALU.add,
            replica_groups=[list(range(NCORES))],
            ins=[cc_in[:, :]], outs=[cc_out[:C, :]],
        )
        nc.sync.dma_start(out=ccs, in_=cc_out[:C, :])
    inv_n = 1.0 / ((1 if SIM1 else NCORES) * NIMG * HW)
    # short critical-path algebra: th' = 0.5*sd/bnw, d' = bnb*sd/bnw - mean
    rbw, nrbw, bbrb = pre
    mean = sb1.tile([C, 1], F32, tag=f"mean{lname}")
    nc.vector.tensor_scalar(mean, ccs[:, 0:1], inv_n, None, op0=ALU.mult)
    msq = sb1.tile([C, 1], F32, tag=f"msq{lname}")
    nc.vector.tensor_tensor(msq, mean, mean, op=ALU.mult)
    var = sb1.tile([C, 1], F32, tag=f"var{lname}")
    nc.vector.scalar_tensor_tensor(var, ccs[:, 1:2], inv_n * SQS, msq,
                                   op0=ALU.mult, op1=ALU.subtract)
    sd = sb1.tile([C, 1], F32, tag=f"sd{lname}")
    nc.scalar.activation(sd, var, AF.Sqrt, bias=eps_t, scale=1.0)
    thp = sb1.tile([C, 1], F32, tag=f"thp{lname}")
    nc.vector.tensor_tensor(thp, sd, rbw, op=ALU.mult)
    dp = sb1.tile([C, 1], F32, tag=f"dp{lname}")
    nc.vector.scalar_tensor_tensor(dp, sd, bbrb, mean,
                                   op0=ALU.mult, op1=ALU.subtract)
    # expand to per-timestep 4^t columns (one op each via pow4 broadcast)
    thp4 = sb1.tile([C, T], F32, tag=f"thp4{lname}")
    nc.vector.tensor_tensor(thp4, thp.broadcast_to([C, T]), pow4,
                            op=ALU.mult)
    dp4 = sb1.tile([C, T], F32, tag=f"dp4{lname}")
    nc.vector.tensor_tensor(dp4, dp.broadcast_to([C, T]), pow4, op=ALU.mult)
    negthp4 = sb1.tile([C, T], F32, tag=f"negthp4{lname}")
    nc.vector.tensor_scalar(negthp4, thp4, -1.0, None, op0=ALU.mult)
    return dp4, thp4, negthp4


def build():
    nc = bacc.Bacc("TRN2", target_bir_lowering=False, debug=False,
                   num_devices=1 if SIM1 else NCORES)

    # --- I/O ---
    x_dt = F32R if L1_MODE == "f32r" else FP16
    x_cols = HWP if L1_MODE == "f32r" else HW
    xhi_d = nc.dram_tensor("xhi", [T, BL, C, x_cols], x_dt,
                           kind="ExternalInput")
    xlo_d = xc8_d = w1c8_d = None
    if L1_MODE == "dr":
        xc8_d = nc.dram_tensor("xc8", [T, BL, C, 2, HW], FP8E5,
                               kind="ExternalInput")
        w1c8_d = nc.dram_tensor("w1c8", [C, 9, 2, C], FP8E5,
                                kind="ExternalInput")
    elif TERMS_L1 >= 2:
        xlo_d = nc.dram_tensor("xlo", [T, BL, C, HW], FP16,
                               kind="ExternalInput")
    w_d = {(1, "hi"): nc.dram_tensor("w1hi", [C, 9, C], x_dt,
                                     kind="ExternalInput")}
    if TERMS_L1 >= 3:
        w_d[(1, "lo")] = nc.dram_tensor("w1lo", [C, 9, C], FP16,
                                        kind="ExternalInput")
    wpk_d = {}
    for l in (2, 3):
        wpk_d[l] = nc.dram_tensor(f"wpk{l}", [C, 9, 2, C], FP8E4,
                                  kind="ExternalInput")
    bn_d = {}
    for l in (1, 2, 3):
        bn_d[(l, "w")] = nc.dram_tensor(f"bnw{l}", [C, 1], F32,
                                        kind="ExternalInput")
        bn_d[(l, "b")] = nc.dram_tensor(f"bnb{l}", [C, 1], F32,
                                        kind="ExternalInput")
    fcw_d = nc.dram_tensor("fcw", [C, 10], F32, kind="ExternalInput")
    fcb_d = nc.dram_tensor("fcb", [1, 10], F32, kind="ExternalInput")
    wvec_d = nc.dram_tensor("wvec", [C, SLOTS], F32, kind="ExternalInput")
    pow4_d = nc.dram_tensor("pow4", [C, T], F32, kind="ExternalInput")
    out_d = nc.dram_tensor("out", [1, BL * 10], F32, kind="ExternalOutput")

    # --- internal DRAM (collective buffers only) ---
    cc_bufs = {}
    for l in (1, 2, 3):
        cc_bufs[l] = (
            nc.dram_tensor(f"cc_in{l}", [C, 2], F32),
            nc.dram_tensor(f"cc_out{l}", [C * (NCORES if AG else 1), 2], F32,
                           addr_space="Shared"),
        )

    with ExitStack() as ctx:
        tc = ctx.enter_context(tile.TileContext(nc))
        sb1 = ctx.enter_context(tc.tile_pool(name="sb1", bufs=1))
        xpool = ctx.enter_context(tc.tile_pool(name="xpool", bufs=4))
        gate_pool = ctx.enter_context(tc.tile_pool(name="gate", bufs=3))
        stage_pool = ctx.enter_context(tc.tile_pool(name="stage", bufs=3))
        mem_pool = ctx.enter_context(tc.tile_pool(name="mem", bufs=1))
        psum_pool = ctx.enter_context(
            tc.tile_pool(name="psum", bufs=(8 * 512) // PSUM_COLS,
                         space="PSUM"))

        # --- load layer-1 weights only; everything else streams during
        # the conv1 window ---
        w_sb = {}
        for key in ((1, "hi"),) + (((1, "lo"),) if (1, "lo") in w_d else ()):
            w_sb[key] = sb1.tile([C, 9, C], x_dt,
                                 name=f"w{key[0]}{key[1]}",
                                 tag=f"w{key[0]}{key[1]}")
            nc.sync.dma_start(out=w_sb[key], in_=w_d[key][:, :, :])
        w1c8_sb = None
        if L1_MODE == "dr":
            w1c8_sb = sb1.tile([C, 9, 2, C], FP8E5, name="w1c8", tag="w1c8")
            nc.sync.dma_start(out=w1c8_sb, in_=w1c8_d[:, :, :, :])

        # Warm-up matmuls: touch each weight tensor once with a fused
        # (self-loading) matmul so the weight-DMA waits attach to a Matmult.
        def warmup_mm(w2d):
            pt = psum_pool.tile([C, PSUM_COLS], F32, tag="pt")
            nc.tensor.matmul(pt[:, 0:8], w2d, w2d[:, 0:8],
                             start=True, stop=True)

        warmup_mm(w_sb[(1, "hi")][:, 4, :])
        if L1_MODE == "dr":
            warmup_mm(w1c8_sb[:, 4, 0, :])

        # persistent SBUF activations: y in f32 (per-image scaled by 4^t)
        ybuf = sb1.tile([C, NIMG, HW], F32, name="ybuf")
        # padded fp8 spike buffers: pad ring stays zero (conv zero-padding);
        # 8 rotating persistent slots (WAR tracked by the tile framework)
        x8bufs = []
        for i in range(8):
            xb = sb1.tile([C, HWP], FP8E4, name=f"x8b{i}", tag=f"x8b{i}")
            nc.vector.memset(xb, 0.0)
            x8bufs.append(xb)

        # L3 pooled spike counts per (t, b)
        poolbuf = sb1.tile([C, BL, T], F32)

        stats = {}
        for l in (1, 2, 3):
            ysum = sb1.tile([C, SLOTS], F32, name=f"ysum{l}",
                            tag=f"sum{l}")
            ysq = sb1.tile([C, SLOTS], F32, name=f"ysq{l}",
                           tag=f"sq{l}")
            stats[l] = (ysum, ysq)
        scratch_pool = ctx.enter_context(tc.tile_pool(name="sqscr", bufs=2))

        # =============== layer 1 conv (standalone window) ===============
        for t in range(T):
            for b in range(BL):
                img = t * BL + b
                xhi = xpool.tile([C, x_cols], x_dt, tag="xin")
                nc.sync.dma_start(out=xhi, in_=xhi_d[t, b, :, :])
                if L1_MODE == "f32r":
                    _conv1_f32r(nc, psum_pool, ybuf[:, img],
                                w_sb[(1, "hi")], xhi,
                                acc=(stats[1][0], stats[1][1], scratch_pool,
                                     NHALF * img),
                                drain_scale=float(4.0 ** t))
                    continue
                xhi3 = xhi.rearrange("c (h w) -> c h w", h=H)
                terms = [(w_sb[(1, "hi")], xhi3)]
                dr_terms = []
                if L1_MODE == "dr":
                    x8 = xpool.tile([C, 2, HW], FP8E5, tag="x8")
                    nc.sync.dma_start(out=x8, in_=xc8_d[t, b])
                    x84 = x8.rearrange("c two (h w) -> c two h w", h=H)
                    dr_terms.append((w1c8_sb, x84))
                else:
                    if TERMS_L1 >= 3:
                        terms.append((w_sb[(1, "lo")], xhi3))
                    if TERMS_L1 >= 2:
                        xlo = xpool.tile([C, HW], FP16, tag="xin")
                        nc.sync.dma_start(out=xlo, in_=xlo_d[t, b, :, :])
                        xlo3 = xlo.rearrange("c (h w) -> c h w", h=H)
                        terms.append((w_sb[(1, "hi")], xlo3))
                _conv_image(nc, psum_pool, ybuf[:, img], terms, dr_terms,
                            acc=(stats[1][0], stats[1][1], scratch_pool,
                                 NHALF * img),
                            drain_scale=float(4.0 ** t))

        # deferred constants: stream in under the conv1 window
        wpk_sb = {}
        for l in (2, 3):
            wpk_sb[l] = sb1.tile([C, 9, 2, C], FP8E4, name=f"wpk{l}",
                                 tag=f"wpk{l}")
            nc.sync.dma_start(out=wpk_sb[l], in_=wpk_d[l][:, :, :, :])
        bn_sb = {}
        for key, dt_ in bn_d.items():
            bn_sb[key] = sb1.tile([C, 1], F32, name=f"bn{key[1]}{key[0]}",
                                  tag=f"bn{key[1]}{key[0]}")
            nc.sync.dma_start(out=bn_sb[key], in_=dt_[:, :])
        fcw_sb = sb1.tile([C, 10], F32)
        nc.sync.dma_start(out=fcw_sb, in_=fcw_d[:, :])
        fcb_sb = sb1.tile([1, 10], F32)
        nc.sync.dma_start(out=fcb_sb, in_=fcb_d[:, :])
        wvec = sb1.tile([C, SLOTS], F32)
        nc.sync.dma_start(out=wvec, in_=wvec_d[:, :])
        pow4 = sb1.tile([C, T], F32)
        nc.sync.dma_start(out=pow4, in_=pow4_d[:, :])
        eps_t = sb1.tile([C, 1], F32)
        nc.vector.memset(eps_t, BN_EPS)
        # off-critical-path per-layer constants: 0.5/bnw, -0.5/bnw, bnb/bnw
        pre = {}
        for l in (1, 2, 3):
            rb = sb1.tile([C, 1], F32, tag=f"rbw{l}")
            nc.vector.reciprocal(out=rb, in_=bn_sb[(l, "w")])
            rbw = sb1.tile([C, 1], F32, tag=f"rbwth{l}")
            nc.vector.tensor_scalar(rbw, rb, THRESH, None, op0=ALU.mult)
            nrbw = sb1.tile([C, 1], F32, tag=f"nrbw{l}")
            nc.vector.tensor_scalar(nrbw, rb, -THRESH, None, op0=ALU.mult)
            bbrb = sb1.tile([C, 1], F32, tag=f"bbrb{l}")
            nc.vector.tensor_tensor(bbrb, bn_sb[(l, "b")], rb, op=ALU.mult)
            pre[l] = (rbw, nrbw, bbrb)
        # fp8 DR warm-ups for the deferred weights (moving data: zeroed x8)
        x8v0 = x8bufs[0].rearrange("c (h w) -> c h w", h=HP)
        for l in (2, 3):
            pt = psum_pool.tile([C, PSUM_COLS], F32, tag="pt")
            nc.tensor.matmul(pt[:, 0:1], wpk_sb[l][:, 4], x8v0[:, 0:2, 0:1],
                             start=True, stop=True,
                             perf_mode=mybir.MatmulPerfMode.DoubleRow)

        dp4_1, thp4_1, _ = _layer_stats(nc, sb1, stats[1], pre[1], eps_t,
                                        wvec, pow4, *cc_bufs[1], "l1")

        # ====== windows 2&3: LIF_l (DVE) interleaved with conv_{l+1} ======
        def lif_conv_window(dp4, thp4, wpk, stats_next):
            """Per image: 4^t-space LIF (r, U', pack) then the DR conv of
            the next layer reading the packed padded spikes."""
            u = mem_pool.tile([C, BL * HW], F32, tag="mem")
            u4 = u.rearrange("c (b p) -> c b p", b=BL)
            for t in range(T):
                # moving fp8 values must stay <= 2^6 (hw e4m3 treats biased
                # exponent 15 as inf/nan): cap the pack scale at 4^6/WS and
                # recover the remaining factor in the drain scale
                pk_scale = float(4.0 ** min(t, 6) / WS)
                ds = float(4.0 ** t / (4.0 ** min(t, 6)))
                for b in range(BL):
                    img = t * BL + b
                    yt = ybuf[:, img]
                    if t == 0:
                        nc.vector.tensor_scalar(u4[:, b], yt,
                                                dp4[:, 0:1], None,
                                                op0=ALU.add)
                    else:
                        # r: U <- U * [U <= thp4[t-1]]  (hard reset)
                        nc.vector.scalar_tensor_tensor(
                            u4[:, b], u4[:, b], thp4[:, t - 1:t], u4[:, b],
                            op0=ALU.is_le, op1=ALU.mult)
                        # U': U <- (y4 + dp4[t]) + U
                        nc.vector.scalar_tensor_tensor(
                            u4[:, b], yt, dp4[:, t:t + 1], u4[:, b],
                            op0=ALU.add, op1=ALU.add)
                    x8 = x8bufs[img % 8]
                    x8int = x8.rearrange("c (h w) -> c h w",
                                         h=HP)[:, 1:H + 1, 1:W + 1]
                    eng = nc.gpsimd if PACK_POOL else nc.vector
                    eng.tensor_scalar(
                        x8int, u4[:, b].rearrange("c (h w) -> c h w", h=H),
                        thp4[:, t:t + 1], pk_scale,
                        op0=ALU.is_gt, op1=ALU.mult)
                    _conv_dr(nc, psum_pool, ybuf[:, img], wpk, x8,
                             acc=(stats_next[0], stats_next[1],
                                  scratch_pool, NHALF * img),
                             drain_scale=ds, y_scale=float(4.0 ** t))

        lif_conv_window(dp4_1, thp4_1, wpk_sb[2], stats[2])
        dp4_2, thp4_2, _ = _layer_stats(nc, sb1, stats[2], pre[2], eps_t,
                                        wvec, pow4, *cc_bufs[2], "l2")

        lif_conv_window(dp4_2, thp4_2, wpk_sb[3], stats[3])
        dp4_3, thp4_3, negthp4_3 = _layer_stats(nc, sb1, stats[3], pre[3],
                                                eps_t, wvec, pow4,
                                                *cc_bufs[3], "l3")

        # =============== layer 3 LIF (4^t space) ===============
        # b 0/1: pure-DVE r-form (reset mask folded into one stt; no
        # cross-engine dependency in the recurrence — the Act Sign is a
        # pooling-only observer).  b 2/3: sign-form with gate+mult on
        # GpSimd (float-scalar ops only; stt is DVE-only in the ISA).
        u = mem_pool.tile([C, BL * HW], F32, tag="mem")
        u4 = u.rearrange("c (b p) -> c b p", b=BL)
        prev_sgn = [None] * BL
        for t in range(T):
            for b in range(BL):
                img = t * BL + b
                yt = ybuf[:, img]
                if t == 0:
                    nc.vector.tensor_scalar(u4[:, b], yt, dp4_3[:, 0:1],
                                            None, op0=ALU.add)
                else:
                    if b < 2:
                        nc.vector.scalar_tensor_tensor(
                            u4[:, b], u4[:, b], thp4_3[:, t - 1:t], u4[:, b],
                            op0=ALU.is_le, op1=ALU.mult)
                    else:
                        gate = gate_pool.tile([C, HW], FP16, tag="gate")
                        nc.gpsimd.tensor_scalar(gate, prev_sgn[b], -0.5, 0.5,
                                                op0=ALU.mult, op1=ALU.add)
                        nc.gpsimd.tensor_tensor(u4[:, b], u4[:, b], gate,
                                                op=ALU.mult)
                    nc.vector.scalar_tensor_tensor(
                        u4[:, b], yt, dp4_3[:, t:t + 1], u4[:, b],
                        op0=ALU.add, op1=ALU.add)
                st = stage_pool.tile([C, HW], FP16, tag="stage")
                nc.scalar.activation(st, u4[:, b], AF.Sign,
                                     bias=negthp4_3[:, t:t + 1], scale=1.0,
                                     accum_out=poolbuf[:, b, t:t + 1])
                prev_sgn[b] = st

        # =============== head: pooling + FC ===============
        feat = sb1.tile([C, BL], F32)
        nc.vector.tensor_reduce(feat.unsqueeze(2), poolbuf,
                                axis=mybir.AxisListType.X, op=ALU.add)
        nc.vector.tensor_scalar(feat, feat, 0.5, T * HW / 2.0,
                                op0=ALU.mult, op1=ALU.add)
        prod = sb1.tile([C, BL, 10], F32)
        nc.vector.tensor_tensor(
            prod, feat.unsqueeze(2).broadcast_to([C, BL, 10]),
            fcw_sb.unsqueeze(1).broadcast_to([C, BL, 10]), op=ALU.mult)
        red = sb1.tile([C, BL, 10], F32)
        nc.gpsimd.partition_all_reduce(red, prod, channels=C,
                                       reduce_op=bass_isa.ReduceOp.add)
        ofin = sb1.tile([1, BL, 10], F32)
        nc.vector.tensor_scalar(ofin, red[0:1], 1.0 / POOL_N, None,
                                op0=ALU.mult)
        nc.vector.tensor_tensor(
            ofin, ofin, fcb_sb.unsqueeze(1).broadcast_to([1, BL, 10]),
            op=ALU.add)
        nc.sync.dma_start(out=out_d[:, :],
                          in_=ofin.rearrange("c b k -> c (b k)"))

        if DBG:
            d_ybuf = nc.dram_tensor("d_ybuf", [C, NIMG * HW], F32,
                                    kind="ExternalOutput")
            nc.sync.dma_start(out=d_ybuf[:, :],
                              in_=ybuf.rearrange("c n p -> c (n p)"))
            d_pool = nc.dram_tensor("d_pool", [C, BL * T], F32,
                                    kind="ExternalOutput")
            nc.sync.dma_start(out=d_pool[:, :],
                              in_=poolbuf.rearrange("c b t -> c (b t)"))
            for l in (1, 2, 3):
                ds = nc.dram_tensor(f"d_sum{l}", [C, SLOTS], F32,
                                    kind="ExternalOutput")
                nc.sync.dma_start(out=ds[:, :], in_=stats[l][0])
                dq = nc.dram_tensor(f"d_sq{l}", [C, SLOTS], F32,
                                    kind="ExternalOutput")
                nc.sync.dma_start(out=dq[:, :], in_=stats[l][1])
            for i in range(8):
                dx8 = nc.dram_tensor(f"d_x8_{i}", [C, HWP], FP8E4,
                                     kind="ExternalOutput")
                nc.sync.dma_start(out=dx8[:, :], in_=x8bufs[i])
            for l, t4 in (("dp1", dp4_1), ("th1", thp4_1),
                          ("dp3", dp4_3), ("nth3", negthp4_3)):
                dt4 = nc.dram_tensor(f"d_{l}", [C, T], F32,
                                     kind="ExternalOutput")
                nc.sync.dma_start(out=dt4[:, :], in_=t4)

    # walrus rejects the standalone InstLdweights this pass splits out for
    # our multi-wait first-of-image matmuls; excess waits lower to
    # event-semaphore chains instead.
    nc.move_matmul_waits_to_ldweights = lambda: None
    nc.compile()
    return nc


_NC_CACHE = {}


def _get_nc():
    if "nc" not in _NC_CACHE:
        _NC_CACHE["nc"] = build()
    return _NC_CACHE["nc"]


def _hi_lo(a):
    hi = a.astype(np.float16)
    lo = (a - hi.astype(np.float32)).astype(np.float16)
    return hi, lo


def _e5m2(a, scale):
    return (np.asarray(a, np.float32) * scale).astype(ml_dtypes.float8_e5m2)


def _e4m3(a):
    return np.asarray(a, np.float32).astype(ml_dtypes.float8_e4m3fn)


def make_in_maps(inp, conv_ws, bns, fc_w, fc_b):
    """Build the 8 per-core input maps from full (numpy) model inputs."""
    common = {}
    for li, w in enumerate(conv_ws, start=1):
        wt = np.ascontiguousarray(
            w.transpose(1, 2, 3, 0).reshape(C, 9, C))  # [I, k, O]
        if li == 1:
            if L1_MODE == "f32r":
                common["w1hi"] = wt.astype(np.float32)
            else:
                hi, lo = _hi_lo(wt)
                common["w1hi"] = hi
            if L1_MODE == "dr":
                w1c8 = np.empty((C, 9, 2, C), dtype=ml_dtypes.float8_e5m2)
                w1c8[:, :, 0, :] = _e5m2(lo.astype(np.float32), DR_S)
                w1c8[:, :, 1, :] = _e5m2(hi.astype(np.float32), 1.0 / DR_S)
                common["w1c8"] = w1c8
            if TERMS_L1 >= 3:
                common["w1lo"] = lo
        else:
            ws_hi = _e4m3(wt * WS)
            ws_lo = _e4m3(wt * WS - ws_hi.astype(np.float32))
            terms = (ws_hi, ws_lo)
            wpk = np.empty((C, 9, 2, C), dtype=ml_dtypes.float8_e4m3fn)
            for m, ((ka, ta), (kb, tb)) in enumerate(PAIRS):
                wpk[:, m, 0, :] = terms[ta][:, ka, :]
                wpk[:, m, 1, :] = terms[tb][:, kb, :]
            common[f"wpk{li}"] = wpk
        common[f"bnw{li}"] = np.ascontiguousarray(
            bns[li - 1][0].reshape(C, 1))
        common[f"bnb{li}"] = np.ascontiguousarray(
            bns[li - 1][1].reshape(C, 1))
    common["fcw"] = np.ascontiguousarray(fc_w.T)          # [C, 10]
    common["fcb"] = np.ascontiguousarray(fc_b.reshape(1, 10))
    wvec = np.zeros((1, SLOTS), np.float32)
    for t in range(T):
        for b in range(BL):
            img = t * BL + b
            for h in range(NHALF):
                wvec[0, NHALF * img + h] = 4.0 ** (-t)
    common["wvec"] = np.ascontiguousarray(np.broadcast_to(wvec, (C, SLOTS)))
    pow4 = np.asarray([[4.0 ** t for t in range(T)]], np.float32)
    common["pow4"] = np.ascontiguousarray(np.broadcast_to(pow4, (C, T)))

    in_maps = []
    for cid in range(NCORES):
        xc = np.ascontiguousarray(
            inp[:, cid * BL:(cid + 1) * BL].reshape(T, BL, C, HW))
        m = dict(common)
        if L1_MODE == "f32r":
            xp = np.zeros((T, BL, C, HP, WP), np.float32)
            xp[:, :, :, 1:H + 1, 1:W + 1] = xc.reshape(T, BL, C, H, W)
            m["xhi"] = xp.reshape(T, BL, C, HWP)
        else:
            xhi, xlo = _hi_lo(xc)
            m["xhi"] = xhi
        if L1_MODE == "dr":
            xc8 = np.empty((T, BL, C, 2, HW), dtype=ml_dtypes.float8_e5m2)
            xc8[:, :, :, 0, :] = _e5m2(xc, 1.0 / DR_S)
            xc8[:, :, :, 1, :] = _e5m2(xlo.astype(np.float32), DR_S)
            m["xc8"] = xc8
        elif TERMS_L1 >= 2:
            m["xlo"] = xlo
        in_maps.append(m)
    return in_maps


def kernel(inp, conv_w1, conv_w2, conv_w3, bn_w1, bn_b1, bn_w2, bn_b2,
           bn_w3, bn_b3, fc_w, fc_b):
    inp = np.asarray(inp, dtype=np.float32)
    ws = [np.asarray(w, dtype=np.float32) for w in (conv_w1, conv_w2, conv_w3)]
    bns = [(np.asarray(bn_w1, np.float32), np.asarray(bn_b1, np.float32)),
           (np.asarray(bn_w2, np.float32), np.asarray(bn_b2, np.float32)),
           (np.asarray(bn_w3, np.float32), np.asarray(bn_b3, np.float32))]
    fc_w = np.asarray(fc_w, np.float32)
    fc_b = np.asarray(fc_b, np.float32)

    nc = _get_nc()
    in_maps = make_in_maps(inp, ws, bns, fc_w, fc_b)
    res = run_bass_kernel_spmd(nc, in_maps, core_ids=list(range(NCORES)))
    out = np.concatenate(
        [r["out"].reshape(BL, 10) for r in res.results], axis=0)
    return out.astype(np.float32)
